# revision 19
# baseline (speedup 1.0000x reference)
"""AlphaRotatedGIoULoss on 8 TRN2 NeuronCores.

Data-parallel: 500000 box pairs sharded 62500/core, laid out as
(128 partitions x 489 boxes). Per-box rotated-GIoU via a branchless
line-integral intersection (slab clipping in each box's axis-aligned
frame + a frame-change correction term), so no sorting/gather is needed.

Restructured v2 (113us session baseline -> ~92us measured, same-p-state;
device DVFS adds ~+20% run-to-run on throttled runs):
- slab roots in center +- half-window form: t = c -+ |Wc*r| with
  pre-negated reciprocal planes, killing the per-edge min/max and subs;
  c23 = dXm*rN - c01 by point symmetry. x and y axes fused into 8u ops.
- cross(corner_e, dir_e)/2 = +-(cross(center,dir)/2) + wh/1024 (the wh
  term is the union's u01 tile), so the full corner planes e2/e3, the
  4SW direction planes, and the 8 ACT copies that built them are gone.
  Everything downstream runs at half-cad scale (final Relu scale 1.0).
- all four reciprocal planes merged into one wide RECIPROCAL_APPROX_FAST
  plus two clamp/cast passes; enclosing-rect x/y stacked into 4u ops.
- the h0 half's line integral is rebased to the moving box's center, so
  its per-edge cad is the constant u01: the whole SA-correction section
  and the per-edge cad/pieces multiplies collapse into
  inter = u01*sum(dt) per half + two h1 cross terms.
- tail packs [union|area_c] into one fast-reciprocal and [iou|rr] into
  one fp16 cube chain; iou/rr clamped to [0,1] so the few near-degenerate
  slab boxes (fp16-saturated reciprocal planes) stay bounded.
Heavy chain is fp16 (DVE 2x mode), geometry pre-scaled by 1/16; scratch
tiles are re-used across phases to stay inside SBUF.
"""
import sys
import numpy as np

for _p in ("/opt/trn_rl_repo", "/root/.axon_site/_ro/trn_rl_repo"):
    if _p not in sys.path:
        sys.path.insert(0, _p)

N_CORES = 8
N_TOTAL = 500000
N_CORE = N_TOTAL // N_CORES   # 62500
P = 128                       # all partitions
FB = 489                      # boxes per partition row (128*489 = 62592)
NPAD = P * FB                 # per-core padded count (92 identity pad boxes)
SW = 2 * FB                   # stacked width (both halves)
PI_2 = 1.5707963267948966
SC = 1.0 / 16.0               # global geometry scale (power of 2, exact)
XQ = 32.0                     # xy fixed-point scale (int16 units = px/32)
XSC = SC / XQ                 # folds the xy dequant into the trig scale
CL = 30000.0                  # fp16-safe clamp for reciprocal planes

_CACHE = {}


def _build():
    import concourse.bass as bass
    import concourse.bacc as bacc
    import concourse.tile as tile
    from concourse import mybir

    f32 = mybir.dt.float32
    f16 = mybir.dt.float16
    i16 = mybir.dt.int16
    AF = mybir.ActivationFunctionType
    OP = mybir.AluOpType
    AXL = mybir.AxisListType
    import os
    debug = bool(os.environ.get("K_DEBUG"))
    nc = bacc.Bacc(None, target_bir_lowering=False)
    ang_d = nc.declare_dram_parameter("ang", [P, 2 * FB], f16, isOutput=False)
    wh_d = nc.declare_dram_parameter("wh", [P, 4 * FB], f16, isOutput=False)
    xy_d = nc.declare_dram_parameter("xy", [P, 4 * FB], i16, isOutput=False)
    out_d = nc.declare_dram_parameter("out", [P, 1], f32, isOutput=True)
    dbg_d = None
    if debug:
        dbg_d = nc.declare_dram_parameter("dbg", [4, P, FB], f32, isOutput=True)

    V = nc.vector
    S = nc.scalar

    def vtt(out, a, b, op):
        V.tensor_tensor(out, a, b, op)

    def vts(out, in_, s1, s2, op0, op1=None):
        if op1 is None:
            V.tensor_scalar(out, in_, s1, None, op0)
        else:
            V.tensor_scalar(out, in_, s1, s2, op0, op1)

    def bce(apv, n=2, axis=1):
        # stride-0 broadcast: insert a [0, n] dim at `axis` (after partition)
        ap_l = [list(d) for d in apv.ap]
        ap_l.insert(axis, [0, n])
        return bass.AP(apv.tensor, apv.offset, ap_l)

    def v2(ap, h=2):
        return ap.rearrange("p (h f) -> p h f", h=h)

    from contextlib import ExitStack

    with tile.TileContext(nc) as tc:
        with (
            tc.tile_pool(name="pre", bufs=1) as pre,
            tc.tile_pool(name="small", bufs=1) as sm,
            ExitStack() as stack,
        ):
            io = stack.enter_context(tc.tile_pool(name="io", bufs=1))
            angT = io.tile([P, 2 * FB], f16, tag="angT")
            whT = io.tile([P, 4 * FB], f16, tag="whT")
            xyT = io.tile([P, 4 * FB], i16, tag="xyT")
            pio2 = sm.tile([P, 1], f32, tag="pio2")
            V.memset(pio2[:], PI_2)
            angV = angT[:].rearrange("p (h f) -> p h f", h=2)
            whV = whT[:].rearrange("p (c f) -> p c f", c=4)   # w1,w2,h1,h2
            xyV = xyT[:].rearrange("p (c f) -> p c f", c=4)   # x1,x2,y1,y2
            # host pre-shuffles inputs into these exact SBUF layouts, so each
            # partition line is one fully-contiguous DMA descriptor.
            # angles first (small, unblocks the Sin chain), then wh, then xy
            nc.sync.dma_start(out=angT[:], in_=ang_d[:])
            nc.sync.dma_start(out=whT[:], in_=wh_d[:])
            nc.sync.dma_start(out=xyT[:], in_=xy_d[:])
            # 1-elem warm-up: loads the Sin ACT table while the DMA runs
            warm = sm.tile([P, 1], f32, tag="warm")
            S.activation(warm[:], pio2[:], AF.Sin)

            def T(name, units, dt=f16):
                # `units` in FB-widths
                return pre.tile([P, units * FB], dt, name=name, tag=name)

            # --- tiles (persistent + phase-reused scratch) ---
            dlt, dltw = T("dlt", 1), T("dltw", 1)
            cdF = T("cdF", 1)                     # cos(dlt), one half
            SDS = T("SDS", 2)                     # [sd | -sd]
            cS, sS = T("cS", 2), T("sS", 2)       # [c2|c1], [s2|s1]
            csS, ssS = T("csS", 2), T("ssS", 2)
            WH = T("WH", 4)                       # [whS | hhS]
            WC2 = T("WC2", 4)                     # [wc | hc]
            WS2 = T("WS2", 4)                     # [ws | hs]
            GN = T("GN", 8)                       # [g0x | n1 | g0y | n2]
            AXY = T("AXY", 8)                     # corners [ax0|ax1|ay0|ay1]
            RP = T("RP", 8, f32)                  # recip staging [wc|hs|ws|hc]
            RN = T("RN", 8)                       # [rNX(e0,e1) | rNY(e0,e1)]
            MW = T("MW", 8)                       # [Wc*rNX | Hc*rNY] -> scratch
            AW = T("AW", 8)                       # |MW|
            CXY = T("CXY", 16)                    # [c01xy | c23xy]
            TLO = T("TLO", 16)                    # tlo; later T0/CAD/PC
            THI = T("THI", 16)                    # thi; later T1/TD/DT
            DXY = T("DXY", 4)                     # [dX | dY]
            DM = T("DM", 4)                       # [2dX | 2dY]
            ddS = T("ddS", 4)                     # [ddx | ddy]
            aP1, aP2 = T("aP1", 4), T("aP2", 4)
            EP = T("EP", 4)                       # [exP | eyP]
            u01 = sm.tile([P, SW], f16, tag="u01")
            union0 = sm.tile([P, FB], f16, tag="union0")
            UA = sm.tile([P, SW], f32, tag="UA")   # [union | area_c]
            area_c = UA[:, FB:SW]

            dXv = DXY[:, 0 * SW:1 * SW]
            dYv = DXY[:, 1 * SW:2 * SW]
            wcF = WC2[:, 0:SW]
            hcF = WC2[:, SW:2 * SW]
            wsF = WS2[:, 0:SW]
            hsF = WS2[:, SW:2 * SW]

            # ---- pre-pass, angle part (only needs angT) ----
            vtt(dlt[:], angV[:, 0], angV[:, 1], OP.subtract)     # a1-a2 (f32)
            S.activation(cS[:, 0:FB], angV[:, 1], AF.Sin, bias=pio2[:])   # c2
            S.activation(cS[:, FB:SW], angV[:, 0], AF.Sin, bias=pio2[:])  # c1
            S.activation(sS[:, 0:FB], angV[:, 1], AF.Sin)                 # s2
            S.activation(sS[:, FB:SW], angV[:, 0], AF.Sin)                # s1
            S.activation(SDS[:, 0:FB], dlt[:], AF.Sin)                    # sd
            S.activation(SDS[:, FB:SW], dlt[:], AF.Sin, scale=-1.0)      # -sd
            # cos(dlt) = sin(dlt + pi/2); wrap into [-pi, pi] first
            V.add_range_wrap(dltw[:], dlt[:], PI_2, 3.141592653589793,
                             6.283185307179586)
            S.activation(cdF[:], dltw[:], AF.Sin)                # cd (1 half)
            # scaled trig copies carry geometry scale + xy dequant into dX/dY
            S.activation(csS[:], cS[:], AF.Copy, scale=XSC)
            S.activation(ssS[:], sS[:], AF.Copy, scale=XSC)

            # ---- pre-pass, wh part ----
            vts(WH[:, 0:SW], whV[:, 0:2], 0.5 * SC, None, OP.mult)       # whS
            vts(WH[:, SW:2 * SW], whV[:, 2:4], 0.5 * SC, None, OP.mult)  # hhS
            WH3 = v2(WH[:])
            # [wc|hc] = [whS|hhS]*cd ; [ws|hs] = [whS|hhS]*sd
            cdb = bass.AP(cdF[:].tensor, cdF[:].offset,
                          [list(cdF[:].ap[0]), [0, 2], [0, 2], [1, FB]])
            WH4 = WH[:].rearrange("p (c h f) -> p c h f", c=2, h=2)
            vtt(WC2[:].rearrange("p (c h f) -> p c h f", c=2, h=2),
                WH4, cdb, OP.mult)
            vtt(v2(WS2[:]), WH3, bce(SDS[:]), OP.mult)
            # corner offsets: g0x = wc-hs, n1 = wc+hs, g0y = ws+hc, n2 = hc-ws
            vtt(GN[:, 0:SW], wcF, hsF, OP.subtract)
            vtt(GN[:, SW:2 * SW], wcF, hsF, OP.add)
            vtt(GN[:, 2 * SW:3 * SW], wsF, hcF, OP.add)
            vtt(GN[:, 3 * SW:4 * SW], hcF, wsF, OP.subtract)
            # clip half-extents [Wc|Hc] = half-swapped views of WH (no ops)
            whp = WH[:].ap[0]
            WHcF = bass.AP(WH[:].tensor, WH[:].offset + FB,
                           [list(whp), [SW, 2], [-FB, 2], [1, FB]])
            WcB = bass.AP(WH[:].tensor, WH[:].offset + FB,
                          [list(whp), [0, 2], [-FB, 2], [1, FB]])
            HcB = bass.AP(WH[:].tensor, WH[:].offset + SW + FB,
                          [list(whp), [0, 2], [-FB, 2], [1, FB]])
            # moving-box bbox half-extents: ex = |wc|+|hs|, ey = |ws|+|hc|
            S.activation(aP1[:], WC2[:], AF.Abs)   # [|wc| | |hc|]
            S.activation(aP2[:], WS2[:], AF.Abs)   # [|ws| | |hs|]
            vtt(EP[:, 0:SW], aP1[:, 0:SW], aP2[:, SW:2 * SW], OP.add)
            vtt(EP[:, SW:2 * SW], aP2[:, 0:SW], aP1[:, SW:2 * SW], OP.add)
            # negated-reciprocal planes rN = -1/d: rNX = [+1/(2wc) | -1/(2hs)],
            # rNY = [+1/(2ws) | +1/(2hc)]; staged f32 as [wc|hs|ws|hc], one
            # wide fast-reciprocal, clamped to +-CL in fp16.
            vts(RP[:, 0:SW], wcF, 2.0, 1e-20, OP.mult, OP.add)
            vts(RP[:, SW:2 * SW], hsF, -2.0, -1e-20, OP.mult, OP.add)
            vts(RP[:, 2 * SW:3 * SW], wsF, 2.0, 1e-20, OP.mult, OP.add)
            vts(RP[:, 3 * SW:4 * SW], hcF, 2.0, 1e-20, OP.mult, OP.add)
            V.reciprocal_approx_fast(out=RP[:], in_=RP[:])
            vts(RN[:], RP[:], CL, -CL, OP.min, OP.max)
            # half-window sizes |Wc*rN| per axis (abs on ACT)
            vtt(MW[:, 0:2 * SW].rearrange("p (e h f) -> p e h f", e=2, h=2),
                WcB, v2(RN[:, 0:2 * SW]).rearrange("p e (h f) -> p e h f",
                                                   h=2), OP.mult)
            vtt(MW[:, 2 * SW:4 * SW].rearrange("p (e h f) -> p e h f",
                                               e=2, h=2),
                HcB, v2(RN[:, 2 * SW:4 * SW]).rearrange("p e (h f) -> p e h f",
                                                        h=2), OP.mult)
            S.activation(AW[:], MW[:], AF.Abs)
            # union0 = (w1h1 + w2h2)/1024; *4 to /256 folded into union STT
            vtt(u01[:], WH[:, 0:SW], WH[:, SW:2 * SW], OP.mult)
            u013 = v2(u01[:])
            vtt(union0[:], u013[:, 0], u013[:, 1], OP.add)

            # ---- pre-pass, xy part (lands last) ----
            dd3 = ddS[:].rearrange("p (c h f) -> p c h f", c=2, h=2)
            ddc = ddS[:].rearrange("p (c f) -> p c f", c=2)
            vtt(dd3[:, 0, 0], xyV[:, 0], xyV[:, 1], OP.subtract)  # x1-x2
            S.activation(dd3[:, 0, 1], dd3[:, 0, 0], AF.Copy, scale=-1.0)
            vtt(dd3[:, 1, 0], xyV[:, 2], xyV[:, 3], OP.subtract)
            S.activation(dd3[:, 1, 1], dd3[:, 1, 0], AF.Copy, scale=-1.0)
            # delta = R^T * (center difference)/16:
            # aP1 = [csS*ddx | csS*ddy], aP2 = [ssS*ddx | ssS*ddy]
            vtt(v2(aP1[:]), bce(csS[:]), ddc, OP.mult)
            vtt(v2(aP2[:]), bce(ssS[:]), ddc, OP.mult)
            vtt(dXv, aP1[:, 0:SW], aP2[:, SW:2 * SW], OP.add)
            vtt(dYv, aP1[:, SW:2 * SW], aP2[:, 0:SW], OP.subtract)
            S.activation(DM[:, 0:SW], dXv, AF.Copy, scale=2.0)
            S.activation(DM[:, SW:2 * SW], dYv, AF.Copy, scale=2.0)

            # corners, edges 0,1 only: ax0 = dX+g0x, ax1 = dX-n1,
            # ay0 = dY+g0y, ay1 = dY+n2
            vtt(AXY[:, 0:SW], dXv, GN[:, 0:SW], OP.add)
            vtt(AXY[:, SW:2 * SW], dXv, GN[:, SW:2 * SW], OP.subtract)
            vtt(AXY[:, 2 * SW:3 * SW], dYv, GN[:, 2 * SW:3 * SW], OP.add)
            vtt(AXY[:, 3 * SW:4 * SW], dYv, GN[:, 3 * SW:4 * SW], OP.add)

            # input tiles no longer needed: free the io pool
            stack.close()

            # ---- slab roots, center form: c01 = ax*rN, c23 = dm*rN - c01 ----
            # CXY = [c01x|c01y | c23x|c23y]; RN/AXY are [x-planes | y-planes]
            vtt(CXY[:, 0:4 * SW], AXY[:], RN[:], OP.mult)
            dmb = bass.AP(DM[:].tensor, DM[:].offset,
                          [list(DM[:].ap[0]), [SW, 2], [0, 2], [1, SW]])
            MM = TLO[:, 0:4 * SW]       # scratch; overwritten by tlo below
            vtt(MM.rearrange("p (a e f) -> p a e f", a=2, e=2), dmb,
                RN[:].rearrange("p (a e f) -> p a e f", a=2, e=2), OP.mult)
            vtt(CXY[:, 4 * SW:8 * SW], CXY[:, 0:4 * SW], MM, OP.subtract)
            # tlo/thi = c -+ aw; AW's [axis|e] layout matches CXY's inner 4SW,
            # broadcast over the e01/e23 pair dim
            awb = bce(AW[:])
            cxy3 = CXY[:].rearrange("p (g f) -> p g f", g=2)
            vtt(v2(TLO[:]), cxy3, awb, OP.subtract)
            vtt(v2(THI[:]), cxy3, awb, OP.add)
            # interval intersect across axes, clamp to [0,1], dt = relu(t1-t0)
            # T0 lives in TLO[0:4SW], T1/TD in THI[0:4SW], DT in THI[4SW:8SW]
            tlo4 = TLO[:].rearrange("p (g a f) -> p g a f", g=2, a=2)
            thi4 = THI[:].rearrange("p (g a f) -> p g a f", g=2, a=2)
            T0 = TLO[:, 0:4 * SW]
            T0v = tlo4[:, :, 0]
            vtt(T0v, tlo4[:, :, 0], tlo4[:, :, 1], OP.max)
            vts(T0v, T0v, 0.0, None, OP.max)
            T1v = thi4[:, :, 0]
            vtt(T1v, thi4[:, :, 0], thi4[:, :, 1], OP.min)
            vts(T1v, T1v, 1.0, None, OP.min)
            vtt(T1v, T1v, T0v, OP.subtract)                   # td in place
            # dt = relu(td) lands contiguous in MW (free after the AW abs)
            DT = MW[:]
            S.activation(v2(DT), T1v, AF.Relu)
            dtg = DT.rearrange("p (g e h f) -> p g e h f", g=2, e=2, h=2)

            # ---- inter via rebased origins: the h0 half's per-edge cad is
            # the constant u01 (origin at the moving box's own center), so
            # inter = u01*sum(dt) per half + the h1 cross terms
            # sum dt over pair and edge dims -> S_dt per (h, box)
            vtt(AXY[:, 0:2 * SW], DT[:, 0:2 * SW], DT[:, 2 * SW:4 * SW],
                OP.add)
            sdt = sm.tile([P, SW], f16, tag="sdt")
            vtt(sdt[:], AXY[:, 0:SW], AXY[:, SW:2 * SW], OP.add)
            bse = sm.tile([P, SW], f16, tag="bse")
            vtt(bse[:], u01[:], sdt[:], OP.mult)
            bse3 = v2(bse[:])
            # h1 cross terms: crA = dY*wc - dX*ws, crBn = dX*hc + dY*hs
            dX_h1 = DXY[:, FB:SW]
            dY_h1 = DXY[:, SW + FB:2 * SW]
            wc_h1 = WC2[:, FB:SW]
            hc_h1 = WC2[:, SW + FB:2 * SW]
            ws_h1 = WS2[:, FB:SW]
            hs_h1 = WS2[:, SW + FB:2 * SW]
            crA = sm.tile([P, FB], f16, tag="crA")
            crBn = sm.tile([P, FB], f16, tag="crBn")
            st1 = sm.tile([P, FB], f16, tag="st1")
            st2 = sm.tile([P, FB], f16, tag="st2")
            vtt(crA[:], dY_h1, wc_h1, OP.mult)
            vtt(st1[:], dX_h1, ws_h1, OP.mult)
            vtt(crA[:], crA[:], st1[:], OP.subtract)
            vtt(crBn[:], dX_h1, hc_h1, OP.mult)
            vtt(st1[:], dY_h1, hs_h1, OP.mult)
            vtt(crBn[:], crBn[:], st1[:], OP.add)
            # du = dt_e0 - dt_e2, dv = dt_e1 - dt_e3 (h1 planes)
            vtt(st1[:], dtg[:, 0, 0, 1], dtg[:, 1, 0, 1], OP.subtract)
            vtt(st2[:], dtg[:, 0, 1, 1], dtg[:, 1, 1, 1], OP.subtract)
            vtt(crA[:], crA[:], st1[:], OP.mult)
            vtt(crBn[:], crBn[:], st2[:], OP.mult)
            inter16 = sm.tile([P, FB], f16, tag="inter16")
            vtt(inter16[:], bse3[:, 0], bse3[:, 1], OP.add)
            vtt(inter16[:], inter16[:], crA[:], OP.add)
            vtt(inter16[:], inter16[:], crBn[:], OP.subtract)

            # ---- enclosing rect (bbox in each frame, min of the two) ----
            # scratch inside CXY (dead after the tlo/thi ops)
            ES1 = CXY[:, 0:2 * SW]
            ES2 = CXY[:, 2 * SW:4 * SW]
            EXT = CXY[:, 4 * SW:6 * SW]
            vtt(ES1, DXY[:, 0:2 * SW], EP[:], OP.add)
            vtt(ES2, EP[:], DXY[:, 0:2 * SW], OP.subtract)
            vtt(v2(ES1, h=2).rearrange("p a (h f) -> p a h f", h=2), 
                v2(ES1, h=2).rearrange("p a (h f) -> p a h f", h=2),
                WHcF, OP.max)
            vtt(v2(ES2, h=2).rearrange("p a (h f) -> p a h f", h=2),
                v2(ES2, h=2).rearrange("p a (h f) -> p a h f", h=2),
                WHcF, OP.max)
            vtt(EXT, ES1, ES2, OP.add)
            exs = sm.tile([P, SW], f16, tag="exs")
            vtt(exs[:], EXT[:, 0:SW], EXT[:, SW:2 * SW], OP.mult)
            es3 = v2(exs[:])
            vtt(area_c, es3[:, 0], es3[:, 1], OP.min)

            inter = sm.tile([P, FB], f32, tag="inter")
            vts(inter[:], inter16[:], 0.0, None, OP.max)  # inter area (/256)

            # ---- final loss (fp32), cubes via one packed ACT Square ----
            fr1 = sm.tile([P, SW], f32, tag="fr1")
            IR = sm.tile([P, SW], f16, tag="IR")       # [iou | rr]
            SQ = sm.tile([P, SW], f16, tag="SQ")
            GU = sm.tile([P, FB], f16, tag="GU")
            lsa = sm.tile([P, 1], f32, tag="lsa")
            union = UA[:, 0:FB]
            # union = 4*union0 - inter  (the *4 restores the /256 scale)
            V.scalar_tensor_tensor(union, union0[:], 4.0, inter[:],
                                   OP.mult, OP.subtract)
            V.reciprocal_approx_fast(out=fr1[:], in_=UA[:])
            vtt(IR[:, 0:FB], inter[:], fr1[:, 0:FB], OP.mult)
            vts(IR[:, 0:FB], IR[:, 0:FB], 1e-6, 1.0, OP.max, OP.min)
            vtt(fr1[:, FB:SW], union, fr1[:, FB:SW], OP.mult)
            vts(fr1[:, FB:SW], fr1[:, FB:SW], 0.0, 1.0, OP.max, OP.min)
            vts(IR[:, FB:SW], fr1[:, FB:SW], -1.0, 1.0, OP.mult, OP.add)
            vtt(SQ[:], IR[:], IR[:], OP.mult)
            vtt(SQ[:], SQ[:], IR[:], OP.mult)                      # cubes
            cb3 = v2(SQ[:])
            vtt(GU[:], cb3[:, 0], cb3[:, 1], OP.subtract)          # giou
            V.tensor_reduce(lsa[:], GU[:], AXL.X, OP.add)          # sum giou
            if debug:
                nc.sync.dma_start(out=dbg_d[0], in_=GU[:])
                nc.sync.dma_start(out=dbg_d[1], in_=inter[:])
                nc.sync.dma_start(out=dbg_d[2], in_=union)
                nc.sync.dma_start(out=dbg_d[3], in_=area_c)
            nc.sync.dma_start(out=out_d[:], in_=lsa[:])

    nc.finalize()
    return nc


def _get_nc():
    if "nc" not in _CACHE:
        _CACHE["nc"] = _build()
    return _CACHE["nc"]


def _repack(pred, target):
    """Per-core input repack: planar rows so every SBUF slice is packed.
    ang/wh in fp16; xy quantized to int16 units of 1/32 px (diffs <= ~1500
    units stay exact in fp16). Rows beyond N_CORE are padded with concentric
    axis-aligned boxes whose giou is exactly 1/64 (subtracted on the host)."""
    in_maps = []
    for i in range(N_CORES):
        sl = slice(i * N_CORE, (i + 1) * N_CORE)
        p, t = pred[sl], target[sl]
        ang = np.zeros((2, NPAD), np.float16)
        ang[0, :N_CORE] = p[:, 4]
        ang[1, :N_CORE] = t[:, 4]
        wh = np.empty((4, NPAD), np.float16)
        wh[0, N_CORE:] = 16.0
        wh[1, N_CORE:] = 8.0
        wh[2, N_CORE:] = 16.0
        wh[3, N_CORE:] = 8.0
        wh[0, :N_CORE] = p[:, 2]
        wh[1, :N_CORE] = t[:, 2]
        wh[2, :N_CORE] = p[:, 3]
        wh[3, :N_CORE] = t[:, 3]
        xy = np.full((4, NPAD), 16384, np.int16)
        for r, col in enumerate((p[:, 0], t[:, 0], p[:, 1], t[:, 1])):
            xy[r, :N_CORE] = np.clip(np.rint(col * XQ), 0, 32767).astype(np.int16)

        def lay(a):
            k = a.shape[0]
            return np.ascontiguousarray(
                a.reshape(k, P, FB).transpose(1, 0, 2).reshape(P, k * FB))
        in_maps.append({"ang": lay(ang), "wh": lay(wh), "xy": lay(xy)})
    return in_maps


def kernel(pred, target):
    from concourse.bass_utils import run_bass_kernel_spmd

    pred = np.ascontiguousarray(np.asarray(pred, dtype=np.float32))
    target = np.ascontiguousarray(np.asarray(target, dtype=np.float32))
    nc = _get_nc()
    in_maps = _repack(pred, target)
    res = run_bass_kernel_spmd(nc, in_maps, core_ids=list(range(N_CORES)))
    gsum = np.float64(0.0)
    for i in range(N_CORES):
        gsum += np.asarray(res.results[i]["out"], dtype=np.float64).sum()
    # subtract the exact giou (=1/64) of the concentric pad boxes
    gsum -= float((NPAD - N_CORE) * N_CORES) * 0.015625
    # loss = mean(1 - giou) = 1 - sum(giou)/N
    return np.float32(1.0 - gsum / N_TOTAL)


# revision 20
# speedup vs baseline: 1.0240x; 1.0240x over previous
"""AlphaRotatedGIoULoss on 8 TRN2 NeuronCores.

Data-parallel: 500000 box pairs sharded 62500/core, laid out as
(128 partitions x 489 boxes). Per-box rotated-GIoU via a branchless
line-integral intersection (slab clipping in each box's axis-aligned
frame + a frame-change correction term), so no sorting/gather is needed.

Restructured v2 (113us session baseline -> ~92us measured, same-p-state;
device DVFS adds ~+20% run-to-run on throttled runs):
- slab roots in center +- half-window form: t = c -+ |Wc*r| with
  pre-negated reciprocal planes, killing the per-edge min/max and subs;
  c23 = dXm*rN - c01 by point symmetry. x and y axes fused into 8u ops.
- cross(corner_e, dir_e)/2 = +-(cross(center,dir)/2) + wh/1024 (the wh
  term is the union's u01 tile), so the full corner planes e2/e3, the
  4SW direction planes, and the 8 ACT copies that built them are gone.
  Everything downstream runs at half-cad scale (final Relu scale 1.0).
- all four reciprocal planes merged into one wide RECIPROCAL_APPROX_FAST
  plus two clamp/cast passes; enclosing-rect x/y stacked into 4u ops.
- the h0 half's line integral is rebased to the moving box's center, so
  its per-edge cad is the constant u01: the whole SA-correction section
  and the per-edge cad/pieces multiplies collapse into
  inter = u01*sum(dt) per half + two h1 cross terms.
- tail packs [union|area_c] into one fast-reciprocal and [iou|rr] into
  one fp16 cube chain; iou/rr clamped to [0,1] so the few near-degenerate
  slab boxes (fp16-saturated reciprocal planes) stay bounded.
Heavy chain is fp16 (DVE 2x mode), geometry pre-scaled by 1/16; scratch
tiles are re-used across phases to stay inside SBUF.
"""
import sys
import numpy as np

for _p in ("/opt/trn_rl_repo", "/root/.axon_site/_ro/trn_rl_repo"):
    if _p not in sys.path:
        sys.path.insert(0, _p)

N_CORES = 8
N_TOTAL = 500000
N_CORE = N_TOTAL // N_CORES   # 62500
P = 128                       # all partitions
FB = 489                      # boxes per partition row (128*489 = 62592)
NPAD = P * FB                 # per-core padded count (92 identity pad boxes)
SW = 2 * FB                   # stacked width (both halves)
PI_2 = 1.5707963267948966
SC = 1.0 / 16.0               # global geometry scale (power of 2, exact)
XQ = 32.0                     # xy fixed-point scale (int16 units = px/32)
XSC = SC / XQ                 # folds the xy dequant into the trig scale
CL = 30000.0                  # fp16-safe clamp for reciprocal planes

_CACHE = {}


def _build():
    import concourse.bass as bass
    import concourse.bacc as bacc
    import concourse.tile as tile
    from concourse import mybir

    f32 = mybir.dt.float32
    f16 = mybir.dt.float16
    i16 = mybir.dt.int16
    AF = mybir.ActivationFunctionType
    OP = mybir.AluOpType
    AXL = mybir.AxisListType
    import os
    debug = bool(os.environ.get("K_DEBUG"))
    nc = bacc.Bacc(None, target_bir_lowering=False)
    ang_d = nc.declare_dram_parameter("ang", [P, 2 * FB], f16, isOutput=False)
    wh_d = nc.declare_dram_parameter("wh", [P, 4 * FB], f16, isOutput=False)
    xy_d = nc.declare_dram_parameter("xy", [P, 4 * FB], i16, isOutput=False)
    out_d = nc.declare_dram_parameter("out", [P, 1], f32, isOutput=True)
    dbg_d = None
    if debug:
        dbg_d = nc.declare_dram_parameter("dbg", [4, P, FB], f32, isOutput=True)

    V = nc.vector
    S = nc.scalar

    def vtt(out, a, b, op):
        V.tensor_tensor(out, a, b, op)

    def vts(out, in_, s1, s2, op0, op1=None):
        if op1 is None:
            V.tensor_scalar(out, in_, s1, None, op0)
        else:
            V.tensor_scalar(out, in_, s1, s2, op0, op1)

    def bce(apv, n=2, axis=1):
        # stride-0 broadcast: insert a [0, n] dim at `axis` (after partition)
        ap_l = [list(d) for d in apv.ap]
        ap_l.insert(axis, [0, n])
        return bass.AP(apv.tensor, apv.offset, ap_l)

    def v2(ap, h=2):
        return ap.rearrange("p (h f) -> p h f", h=h)

    from contextlib import ExitStack

    with tile.TileContext(nc) as tc:
        with (
            tc.tile_pool(name="pre", bufs=1) as pre,
            tc.tile_pool(name="small", bufs=1) as sm,
            ExitStack() as stack,
        ):
            io = stack.enter_context(tc.tile_pool(name="io", bufs=1))
            angT = io.tile([P, 2 * FB], f16, tag="angT")
            whT = io.tile([P, 4 * FB], f16, tag="whT")
            xyT = io.tile([P, 4 * FB], i16, tag="xyT")
            pio2 = sm.tile([P, 1], f32, tag="pio2")
            V.memset(pio2[:], PI_2)
            angV = angT[:].rearrange("p (h f) -> p h f", h=2)
            whV = whT[:].rearrange("p (c f) -> p c f", c=4)   # w1,w2,h1,h2
            xyV = xyT[:].rearrange("p (c f) -> p c f", c=4)   # x1,x2,y1,y2
            # host pre-shuffles inputs into these exact SBUF layouts, so each
            # partition line is one fully-contiguous DMA descriptor.
            # angles first (small, unblocks the Sin chain), then wh, then xy
            nc.sync.dma_start(out=angT[:], in_=ang_d[:])
            nc.sync.dma_start(out=whT[:], in_=wh_d[:])
            nc.sync.dma_start(out=xyT[:], in_=xy_d[:])
            # 1-elem warm-up: loads the Sin ACT table while the DMA runs
            warm = sm.tile([P, 1], f32, tag="warm")
            S.activation(warm[:], pio2[:], AF.Sin)

            def T(name, units, dt=f16):
                # `units` in FB-widths
                return pre.tile([P, units * FB], dt, name=name, tag=name)

            # --- tiles (persistent + phase-reused scratch) ---
            dlt, dltw = T("dlt", 1), T("dltw", 1)
            cdF = T("cdF", 1)                     # cos(dlt), one half
            SDS = T("SDS", 2)                     # [sd | -sd]
            cS, sS = T("cS", 2), T("sS", 2)       # [c2|c1], [s2|s1]
            csS, ssS = T("csS", 2), T("ssS", 2)
            WH = T("WH", 4)                       # [whS | hhS]
            WC2 = T("WC2", 4)                     # [wc | hc]
            WS2 = T("WS2", 4)                     # [ws | hs]
            GN = T("GN", 8)                       # [g0x | n1 | g0y | n2]
            AXY = T("AXY", 8)                     # corners [ax0|ax1|ay0|ay1]
            RP = T("RP", 8, f32)                  # recip staging [wc|hs|ws|hc]
            RN = T("RN", 8)                       # [rNX(e0,e1) | rNY(e0,e1)]
            MW = T("MW", 8)                       # [Wc*rNX | Hc*rNY] -> scratch
            AW = T("AW", 8)                       # |MW|
            CXY = T("CXY", 16)                    # [c01xy | c23xy]
            TLO = T("TLO", 16)                    # tlo; later T0/CAD/PC
            THI = T("THI", 16)                    # thi; later T1/TD/DT
            DXY = T("DXY", 4)                     # [dX | dY]
            DM = T("DM", 4)                       # [2dX | 2dY]
            ddS = T("ddS", 4)                     # [ddx | ddy]
            aP1, aP2 = T("aP1", 4), T("aP2", 4)
            EP = T("EP", 4)                       # [exP | eyP]
            u01 = sm.tile([P, SW], f16, tag="u01")
            union0 = sm.tile([P, FB], f16, tag="union0")
            UA = sm.tile([P, SW], f32, tag="UA")   # [union | area_c]
            area_c = UA[:, FB:SW]

            dXv = DXY[:, 0 * SW:1 * SW]
            dYv = DXY[:, 1 * SW:2 * SW]
            wcF = WC2[:, 0:SW]
            hcF = WC2[:, SW:2 * SW]
            wsF = WS2[:, 0:SW]
            hsF = WS2[:, SW:2 * SW]

            # ---- pre-pass, angle part (only needs angT) ----
            vtt(dlt[:], angV[:, 0], angV[:, 1], OP.subtract)     # a1-a2 (f32)
            S.activation(cS[:, 0:FB], angV[:, 1], AF.Sin, bias=pio2[:])   # c2
            S.activation(cS[:, FB:SW], angV[:, 0], AF.Sin, bias=pio2[:])  # c1
            S.activation(sS[:, 0:FB], angV[:, 1], AF.Sin)                 # s2
            S.activation(sS[:, FB:SW], angV[:, 0], AF.Sin)                # s1
            S.activation(SDS[:, 0:FB], dlt[:], AF.Sin)                    # sd
            S.activation(SDS[:, FB:SW], dlt[:], AF.Sin, scale=-1.0)      # -sd
            # cos(dlt) = sin(dlt + pi/2); wrap into [-pi, pi] first
            V.add_range_wrap(dltw[:], dlt[:], PI_2, 3.141592653589793,
                             6.283185307179586)
            S.activation(cdF[:], dltw[:], AF.Sin)                # cd (1 half)
            # scaled trig copies carry geometry scale + xy dequant into dX/dY
            S.activation(csS[:], cS[:], AF.Copy, scale=XSC)
            S.activation(ssS[:], sS[:], AF.Copy, scale=XSC)

            # ---- pre-pass, wh part ----
            vts(WH[:, 0:SW], whV[:, 0:2], 0.5 * SC, None, OP.mult)       # whS
            vts(WH[:, SW:2 * SW], whV[:, 2:4], 0.5 * SC, None, OP.mult)  # hhS
            WH3 = v2(WH[:])
            # [wc|hc] = [whS|hhS]*cd ; [ws|hs] = [whS|hhS]*sd
            cdb = bass.AP(cdF[:].tensor, cdF[:].offset,
                          [list(cdF[:].ap[0]), [0, 2], [0, 2], [1, FB]])
            WH4 = WH[:].rearrange("p (c h f) -> p c h f", c=2, h=2)
            vtt(WC2[:].rearrange("p (c h f) -> p c h f", c=2, h=2),
                WH4, cdb, OP.mult)
            vtt(v2(WS2[:]), WH3, bce(SDS[:]), OP.mult)
            # corner offsets: g0x = wc-hs, n1 = wc+hs, g0y = ws+hc, n2 = hc-ws
            vtt(GN[:, 0:SW], wcF, hsF, OP.subtract)
            vtt(GN[:, SW:2 * SW], wcF, hsF, OP.add)
            vtt(GN[:, 2 * SW:3 * SW], wsF, hcF, OP.add)
            vtt(GN[:, 3 * SW:4 * SW], hcF, wsF, OP.subtract)
            # clip half-extents [Wc|Hc] = half-swapped views of WH (no ops)
            whp = WH[:].ap[0]
            WHcF = bass.AP(WH[:].tensor, WH[:].offset + FB,
                           [list(whp), [SW, 2], [-FB, 2], [1, FB]])
            WcB = bass.AP(WH[:].tensor, WH[:].offset + FB,
                          [list(whp), [0, 2], [-FB, 2], [1, FB]])
            HcB = bass.AP(WH[:].tensor, WH[:].offset + SW + FB,
                          [list(whp), [0, 2], [-FB, 2], [1, FB]])
            # moving-box bbox half-extents: ex = |wc|+|hs|, ey = |ws|+|hc|
            S.activation(aP1[:], WC2[:], AF.Abs)   # [|wc| | |hc|]
            S.activation(aP2[:], WS2[:], AF.Abs)   # [|ws| | |hs|]
            vtt(EP[:, 0:SW], aP1[:, 0:SW], aP2[:, SW:2 * SW], OP.add)
            vtt(EP[:, SW:2 * SW], aP2[:, 0:SW], aP1[:, SW:2 * SW], OP.add)
            # negated-reciprocal planes rN = -1/d: rNX = [+1/(2wc) | -1/(2hs)],
            # rNY = [+1/(2ws) | +1/(2hc)]; staged f32 as [wc|hs|ws|hc], one
            # wide fast-reciprocal, clamped to +-CL in fp16.
            vts(RP[:, 0:SW], wcF, 2.0, 1e-20, OP.mult, OP.add)
            vts(RP[:, SW:2 * SW], hsF, -2.0, -1e-20, OP.mult, OP.add)
            vts(RP[:, 2 * SW:3 * SW], wsF, 2.0, 1e-20, OP.mult, OP.add)
            vts(RP[:, 3 * SW:4 * SW], hcF, 2.0, 1e-20, OP.mult, OP.add)
            V.reciprocal_approx_fast(out=RP[:], in_=RP[:])
            vts(RN[:], RP[:], CL, -CL, OP.min, OP.max)
            # half-window sizes |Wc*rN| per axis (abs on ACT)
            vtt(MW[:, 0:2 * SW].rearrange("p (e h f) -> p e h f", e=2, h=2),
                WcB, v2(RN[:, 0:2 * SW]).rearrange("p e (h f) -> p e h f",
                                                   h=2), OP.mult)
            vtt(MW[:, 2 * SW:4 * SW].rearrange("p (e h f) -> p e h f",
                                               e=2, h=2),
                HcB, v2(RN[:, 2 * SW:4 * SW]).rearrange("p e (h f) -> p e h f",
                                                        h=2), OP.mult)
            S.activation(AW[:], MW[:], AF.Abs)
            # union0 = (w1h1 + w2h2)/1024; *4 to /256 folded into union STT
            vtt(u01[:], WH[:, 0:SW], WH[:, SW:2 * SW], OP.mult)
            u013 = v2(u01[:])
            vtt(union0[:], u013[:, 0], u013[:, 1], OP.add)

            # ---- pre-pass, xy part (lands last) ----
            dd3 = ddS[:].rearrange("p (c h f) -> p c h f", c=2, h=2)
            ddc = ddS[:].rearrange("p (c f) -> p c f", c=2)
            vtt(dd3[:, 0, 0], xyV[:, 0], xyV[:, 1], OP.subtract)  # x1-x2
            S.activation(dd3[:, 0, 1], dd3[:, 0, 0], AF.Copy, scale=-1.0)
            vtt(dd3[:, 1, 0], xyV[:, 2], xyV[:, 3], OP.subtract)
            S.activation(dd3[:, 1, 1], dd3[:, 1, 0], AF.Copy, scale=-1.0)
            # delta = R^T * (center difference)/16:
            # aP1 = [csS*ddx | csS*ddy], aP2 = [ssS*ddx | ssS*ddy]
            vtt(v2(aP1[:]), bce(csS[:]), ddc, OP.mult)
            vtt(v2(aP2[:]), bce(ssS[:]), ddc, OP.mult)
            vtt(dXv, aP1[:, 0:SW], aP2[:, SW:2 * SW], OP.add)
            vtt(dYv, aP1[:, SW:2 * SW], aP2[:, 0:SW], OP.subtract)
            S.activation(DM[:, 0:SW], dXv, AF.Copy, scale=2.0)
            S.activation(DM[:, SW:2 * SW], dYv, AF.Copy, scale=2.0)

            # corners, edges 0,1 only: ax0 = dX+g0x, ax1 = dX-n1,
            # ay0 = dY+g0y, ay1 = dY+n2
            vtt(AXY[:, 0:SW], dXv, GN[:, 0:SW], OP.add)
            vtt(AXY[:, SW:2 * SW], dXv, GN[:, SW:2 * SW], OP.subtract)
            vtt(AXY[:, 2 * SW:3 * SW], dYv, GN[:, 2 * SW:3 * SW], OP.add)
            vtt(AXY[:, 3 * SW:4 * SW], dYv, GN[:, 3 * SW:4 * SW], OP.add)

            # input tiles no longer needed: free the io pool
            stack.close()

            # ---- slab roots, center form: c01 = ax*rN; the e23 interval
            # bounds derive from the e01 bounds by the point-symmetry shift:
            # tlo23 = tlo01 - mm, thi23 = thi01 - mm (mm = 2dX*rN), so c23
            # is never materialized and aw is applied once.
            vtt(CXY[:, 0:4 * SW], AXY[:], RN[:], OP.mult)
            dmb = bass.AP(DM[:].tensor, DM[:].offset,
                          [list(DM[:].ap[0]), [SW, 2], [0, 2], [1, SW]])
            MM = CXY[:, 4 * SW:8 * SW]
            vtt(MM.rearrange("p (a e f) -> p a e f", a=2, e=2), dmb,
                RN[:].rearrange("p (a e f) -> p a e f", a=2, e=2), OP.mult)
            vtt(TLO[:, 0:4 * SW], CXY[:, 0:4 * SW], AW[:], OP.subtract)
            vtt(THI[:, 0:4 * SW], CXY[:, 0:4 * SW], AW[:], OP.add)
            vtt(TLO[:, 4 * SW:8 * SW], TLO[:, 0:4 * SW], MM, OP.subtract)
            vtt(THI[:, 4 * SW:8 * SW], THI[:, 0:4 * SW], MM, OP.subtract)
            # interval intersect across axes, clamp to [0,1], dt = relu(t1-t0)
            # T0 lives in TLO[0:4SW], T1/TD in THI[0:4SW], DT in THI[4SW:8SW]
            tlo4 = TLO[:].rearrange("p (g a f) -> p g a f", g=2, a=2)
            thi4 = THI[:].rearrange("p (g a f) -> p g a f", g=2, a=2)
            T0 = TLO[:, 0:4 * SW]
            T0v = tlo4[:, :, 0]
            vtt(T0v, tlo4[:, :, 0], tlo4[:, :, 1], OP.max)
            vts(T0v, T0v, 0.0, None, OP.max)
            T1v = thi4[:, :, 0]
            vtt(T1v, thi4[:, :, 0], thi4[:, :, 1], OP.min)
            vts(T1v, T1v, 1.0, None, OP.min)
            vtt(T1v, T1v, T0v, OP.subtract)                   # td in place
            # dt = relu(td) lands contiguous in MW (free after the AW abs)
            DT = MW[:]
            S.activation(v2(DT), T1v, AF.Relu)
            dtg = DT.rearrange("p (g e h f) -> p g e h f", g=2, e=2, h=2)

            # ---- inter via rebased origins: the h0 half's per-edge cad is
            # the constant u01 (origin at the moving box's own center), so
            # inter = u01*sum(dt) per half + the h1 cross terms
            # sum dt over pair and edge dims -> S_dt per (h, box)
            vtt(AXY[:, 0:2 * SW], DT[:, 0:2 * SW], DT[:, 2 * SW:4 * SW],
                OP.add)
            sdt = sm.tile([P, SW], f16, tag="sdt")
            vtt(sdt[:], AXY[:, 0:SW], AXY[:, SW:2 * SW], OP.add)
            bse = sm.tile([P, SW], f16, tag="bse")
            vtt(bse[:], u01[:], sdt[:], OP.mult)
            bse3 = v2(bse[:])
            # h1 cross terms: crA = dY*wc - dX*ws, crBn = dX*hc + dY*hs
            dX_h1 = DXY[:, FB:SW]
            dY_h1 = DXY[:, SW + FB:2 * SW]
            wc_h1 = WC2[:, FB:SW]
            hc_h1 = WC2[:, SW + FB:2 * SW]
            ws_h1 = WS2[:, FB:SW]
            hs_h1 = WS2[:, SW + FB:2 * SW]
            crA = sm.tile([P, FB], f16, tag="crA")
            crBn = sm.tile([P, FB], f16, tag="crBn")
            st1 = sm.tile([P, FB], f16, tag="st1")
            st2 = sm.tile([P, FB], f16, tag="st2")
            vtt(crA[:], dY_h1, wc_h1, OP.mult)
            vtt(st1[:], dX_h1, ws_h1, OP.mult)
            vtt(crA[:], crA[:], st1[:], OP.subtract)
            vtt(crBn[:], dX_h1, hc_h1, OP.mult)
            vtt(st1[:], dY_h1, hs_h1, OP.mult)
            vtt(crBn[:], crBn[:], st1[:], OP.add)
            # du = dt_e0 - dt_e2, dv = dt_e1 - dt_e3 (h1 planes)
            vtt(st1[:], dtg[:, 0, 0, 1], dtg[:, 1, 0, 1], OP.subtract)
            vtt(st2[:], dtg[:, 0, 1, 1], dtg[:, 1, 1, 1], OP.subtract)
            vtt(crA[:], crA[:], st1[:], OP.mult)
            vtt(crBn[:], crBn[:], st2[:], OP.mult)
            inter16 = sm.tile([P, FB], f16, tag="inter16")
            vtt(inter16[:], bse3[:, 0], bse3[:, 1], OP.add)
            vtt(inter16[:], inter16[:], crA[:], OP.add)
            vtt(inter16[:], inter16[:], crBn[:], OP.subtract)

            # ---- enclosing rect (bbox in each frame, min of the two) ----
            # scratch inside CXY (dead after the tlo/thi ops)
            ES1 = CXY[:, 0:2 * SW]
            ES2 = CXY[:, 2 * SW:4 * SW]
            EXT = CXY[:, 4 * SW:6 * SW]
            vtt(ES1, DXY[:, 0:2 * SW], EP[:], OP.add)
            vtt(ES2, EP[:], DXY[:, 0:2 * SW], OP.subtract)
            vtt(v2(ES1, h=2).rearrange("p a (h f) -> p a h f", h=2), 
                v2(ES1, h=2).rearrange("p a (h f) -> p a h f", h=2),
                WHcF, OP.max)
            vtt(v2(ES2, h=2).rearrange("p a (h f) -> p a h f", h=2),
                v2(ES2, h=2).rearrange("p a (h f) -> p a h f", h=2),
                WHcF, OP.max)
            vtt(EXT, ES1, ES2, OP.add)
            exs = sm.tile([P, SW], f16, tag="exs")
            vtt(exs[:], EXT[:, 0:SW], EXT[:, SW:2 * SW], OP.mult)
            es3 = v2(exs[:])
            vtt(area_c, es3[:, 0], es3[:, 1], OP.min)

            inter = sm.tile([P, FB], f32, tag="inter")
            vts(inter[:], inter16[:], 0.0, None, OP.max)  # inter area (/256)

            # ---- final loss (fp32), cubes via one packed ACT Square ----
            fr1 = sm.tile([P, SW], f32, tag="fr1")
            IR = sm.tile([P, SW], f16, tag="IR")       # [iou | rr]
            SQ = sm.tile([P, SW], f16, tag="SQ")
            GU = sm.tile([P, FB], f16, tag="GU")
            lsa = sm.tile([P, 1], f32, tag="lsa")
            union = UA[:, 0:FB]
            # union = 4*union0 - inter  (the *4 restores the /256 scale)
            V.scalar_tensor_tensor(union, union0[:], 4.0, inter[:],
                                   OP.mult, OP.subtract)
            V.reciprocal_approx_fast(out=fr1[:], in_=UA[:])
            vtt(IR[:, 0:FB], inter[:], fr1[:, 0:FB], OP.mult)
            vts(IR[:, 0:FB], IR[:, 0:FB], 1e-6, 1.0, OP.max, OP.min)
            vtt(fr1[:, FB:SW], union, fr1[:, FB:SW], OP.mult)
            vts(fr1[:, FB:SW], fr1[:, FB:SW], 0.0, 1.0, OP.max, OP.min)
            vts(IR[:, FB:SW], fr1[:, FB:SW], -1.0, 1.0, OP.mult, OP.add)
            vtt(SQ[:], IR[:], IR[:], OP.mult)
            vtt(SQ[:], SQ[:], IR[:], OP.mult)                      # cubes
            cb3 = v2(SQ[:])
            vtt(GU[:], cb3[:, 0], cb3[:, 1], OP.subtract)          # giou
            V.tensor_reduce(lsa[:], GU[:], AXL.X, OP.add)          # sum giou
            if debug:
                nc.sync.dma_start(out=dbg_d[0], in_=GU[:])
                nc.sync.dma_start(out=dbg_d[1], in_=inter[:])
                nc.sync.dma_start(out=dbg_d[2], in_=union)
                nc.sync.dma_start(out=dbg_d[3], in_=area_c)
            nc.sync.dma_start(out=out_d[:], in_=lsa[:])

    nc.finalize()
    return nc


def _get_nc():
    if "nc" not in _CACHE:
        _CACHE["nc"] = _build()
    return _CACHE["nc"]


def _repack(pred, target):
    """Per-core input repack: planar rows so every SBUF slice is packed.
    ang/wh in fp16; xy quantized to int16 units of 1/32 px (diffs <= ~1500
    units stay exact in fp16). Rows beyond N_CORE are padded with concentric
    axis-aligned boxes whose giou is exactly 1/64 (subtracted on the host)."""
    in_maps = []
    for i in range(N_CORES):
        sl = slice(i * N_CORE, (i + 1) * N_CORE)
        p, t = pred[sl], target[sl]
        ang = np.zeros((2, NPAD), np.float16)
        ang[0, :N_CORE] = p[:, 4]
        ang[1, :N_CORE] = t[:, 4]
        wh = np.empty((4, NPAD), np.float16)
        wh[0, N_CORE:] = 16.0
        wh[1, N_CORE:] = 8.0
        wh[2, N_CORE:] = 16.0
        wh[3, N_CORE:] = 8.0
        wh[0, :N_CORE] = p[:, 2]
        wh[1, :N_CORE] = t[:, 2]
        wh[2, :N_CORE] = p[:, 3]
        wh[3, :N_CORE] = t[:, 3]
        xy = np.full((4, NPAD), 16384, np.int16)
        for r, col in enumerate((p[:, 0], t[:, 0], p[:, 1], t[:, 1])):
            xy[r, :N_CORE] = np.clip(np.rint(col * XQ), 0, 32767).astype(np.int16)

        def lay(a):
            k = a.shape[0]
            return np.ascontiguousarray(
                a.reshape(k, P, FB).transpose(1, 0, 2).reshape(P, k * FB))
        in_maps.append({"ang": lay(ang), "wh": lay(wh), "xy": lay(xy)})
    return in_maps


def kernel(pred, target):
    from concourse.bass_utils import run_bass_kernel_spmd

    pred = np.ascontiguousarray(np.asarray(pred, dtype=np.float32))
    target = np.ascontiguousarray(np.asarray(target, dtype=np.float32))
    nc = _get_nc()
    in_maps = _repack(pred, target)
    res = run_bass_kernel_spmd(nc, in_maps, core_ids=list(range(N_CORES)))
    gsum = np.float64(0.0)
    for i in range(N_CORES):
        gsum += np.asarray(res.results[i]["out"], dtype=np.float64).sum()
    # subtract the exact giou (=1/64) of the concentric pad boxes
    gsum -= float((NPAD - N_CORE) * N_CORES) * 0.015625
    # loss = mean(1 - giou) = 1 - sum(giou)/N
    return np.float32(1.0 - gsum / N_TOTAL)


# revision 21
# speedup vs baseline: 1.0245x; 1.0005x over previous
"""AlphaRotatedGIoULoss on 8 TRN2 NeuronCores.

Data-parallel: 500000 box pairs sharded 62500/core, laid out as
(128 partitions x 489 boxes). Per-box rotated-GIoU via a branchless
line-integral intersection (slab clipping in each box's axis-aligned
frame + a frame-change correction term), so no sorting/gather is needed.

Restructured v2 (113us session baseline -> 87.8-89.4us, same-p-state;
device DVFS adds ~+20% run-to-run on throttled runs):
- slab roots in center +- half-window form: t = c -+ |Wc*r| with
  pre-negated reciprocal planes, killing the per-edge min/max and subs;
  the e23 interval bounds derive from the e01 bounds by the point-symmetry
  shift (tlo23 = tlo01 - mm), so c23 is never materialized and the
  half-window is applied once. x and y axes fused into 8u ops.
- cross(corner_e, dir_e)/2 = +-(cross(center,dir)/2) + wh/1024 (the wh
  term is the union's u01 tile), so the full corner planes e2/e3, the
  4SW direction planes, and the 8 ACT copies that built them are gone.
  Everything downstream runs at half-cad scale (final Relu scale 1.0).
- all four reciprocal planes merged into one wide RECIPROCAL_APPROX_FAST
  plus two clamp/cast passes; enclosing-rect x/y stacked into 4u ops.
- the h0 half's line integral is rebased to the moving box's center, so
  its per-edge cad is the constant u01: the whole SA-correction section
  and the per-edge cad/pieces multiplies collapse into
  inter = u01*sum(dt) per half + two h1 cross terms.
- tail packs [union|area_c] into one fast-reciprocal and [iou|rr] into
  one fp16 cube chain; iou/rr clamped to [0,1] so the few near-degenerate
  slab boxes (fp16-saturated reciprocal planes) stay bounded.
Heavy chain is fp16 (DVE 2x mode), geometry pre-scaled by 1/16; scratch
tiles are re-used across phases to stay inside SBUF.
"""
import sys
import numpy as np

for _p in ("/opt/trn_rl_repo", "/root/.axon_site/_ro/trn_rl_repo"):
    if _p not in sys.path:
        sys.path.insert(0, _p)

N_CORES = 8
N_TOTAL = 500000
N_CORE = N_TOTAL // N_CORES   # 62500
P = 128                       # all partitions
FB = 489                      # boxes per partition row (128*489 = 62592)
NPAD = P * FB                 # per-core padded count (92 identity pad boxes)
SW = 2 * FB                   # stacked width (both halves)
PI_2 = 1.5707963267948966
SC = 1.0 / 16.0               # global geometry scale (power of 2, exact)
XQ = 32.0                     # xy fixed-point scale (int16 units = px/32)
XSC = SC / XQ                 # folds the xy dequant into the trig scale
CL = 30000.0                  # fp16-safe clamp for reciprocal planes

_CACHE = {}


def _build():
    import concourse.bass as bass
    import concourse.bacc as bacc
    import concourse.tile as tile
    from concourse import mybir

    f32 = mybir.dt.float32
    f16 = mybir.dt.float16
    i16 = mybir.dt.int16
    AF = mybir.ActivationFunctionType
    OP = mybir.AluOpType
    AXL = mybir.AxisListType
    import os
    debug = bool(os.environ.get("K_DEBUG"))
    nc = bacc.Bacc(None, target_bir_lowering=False)
    ang_d = nc.declare_dram_parameter("ang", [P, 2 * FB], f16, isOutput=False)
    wh_d = nc.declare_dram_parameter("wh", [P, 4 * FB], f16, isOutput=False)
    xy_d = nc.declare_dram_parameter("xy", [P, 4 * FB], i16, isOutput=False)
    out_d = nc.declare_dram_parameter("out", [P, 1], f32, isOutput=True)
    dbg_d = None
    if debug:
        dbg_d = nc.declare_dram_parameter("dbg", [4, P, FB], f32, isOutput=True)

    V = nc.vector
    S = nc.scalar

    def vtt(out, a, b, op):
        V.tensor_tensor(out, a, b, op)

    def vts(out, in_, s1, s2, op0, op1=None):
        if op1 is None:
            V.tensor_scalar(out, in_, s1, None, op0)
        else:
            V.tensor_scalar(out, in_, s1, s2, op0, op1)

    def bce(apv, n=2, axis=1):
        # stride-0 broadcast: insert a [0, n] dim at `axis` (after partition)
        ap_l = [list(d) for d in apv.ap]
        ap_l.insert(axis, [0, n])
        return bass.AP(apv.tensor, apv.offset, ap_l)

    def v2(ap, h=2):
        return ap.rearrange("p (h f) -> p h f", h=h)

    from contextlib import ExitStack

    with tile.TileContext(nc) as tc:
        with (
            tc.tile_pool(name="pre", bufs=1) as pre,
            tc.tile_pool(name="small", bufs=1) as sm,
            ExitStack() as stack,
        ):
            io = stack.enter_context(tc.tile_pool(name="io", bufs=1))
            angT = io.tile([P, 2 * FB], f16, tag="angT")
            whT = io.tile([P, 4 * FB], f16, tag="whT")
            xyT = io.tile([P, 4 * FB], i16, tag="xyT")
            pio2 = sm.tile([P, 1], f32, tag="pio2")
            V.memset(pio2[:], PI_2)
            angV = angT[:].rearrange("p (h f) -> p h f", h=2)
            whV = whT[:].rearrange("p (c f) -> p c f", c=4)   # w1,w2,h1,h2
            xyV = xyT[:].rearrange("p (c f) -> p c f", c=4)   # x1,x2,y1,y2
            # host pre-shuffles inputs into these exact SBUF layouts, so each
            # partition line is one fully-contiguous DMA descriptor.
            # angles first (small, unblocks the Sin chain), then wh, then xy
            nc.sync.dma_start(out=angT[:], in_=ang_d[:])
            nc.sync.dma_start(out=whT[:], in_=wh_d[:])
            nc.sync.dma_start(out=xyT[:], in_=xy_d[:])
            # 1-elem warm-up: loads the Sin ACT table while the DMA runs
            warm = sm.tile([P, 1], f32, tag="warm")
            S.activation(warm[:], pio2[:], AF.Sin)

            def T(name, units, dt=f16):
                # `units` in FB-widths
                return pre.tile([P, units * FB], dt, name=name, tag=name)

            # --- tiles (persistent + phase-reused scratch) ---
            dlt, dltw = T("dlt", 1), T("dltw", 1)
            cdF = T("cdF", 1)                     # cos(dlt), one half
            SDS = T("SDS", 2)                     # [sd | -sd]
            cS, sS = T("cS", 2), T("sS", 2)       # [c2|c1], [s2|s1]
            csS, ssS = T("csS", 2), T("ssS", 2)
            WH = T("WH", 4)                       # [whS | hhS]
            WC2 = T("WC2", 4)                     # [wc | hc]
            WS2 = T("WS2", 4)                     # [ws | hs]
            GN = T("GN", 8)                       # [g0x | n1 | g0y | n2]
            AXY = T("AXY", 8)                     # corners [ax0|ax1|ay0|ay1]
            RP = T("RP", 8, f32)                  # recip staging [wc|hs|ws|hc]
            RN = T("RN", 8)                       # [rNX(e0,e1) | rNY(e0,e1)]
            MW = T("MW", 8)                       # [Wc*rNX | Hc*rNY] -> scratch
            AW = T("AW", 8)                       # |MW|
            CXY = T("CXY", 16)                    # [c01xy | c23xy]
            TLO = T("TLO", 16)                    # tlo; later T0/CAD/PC
            THI = T("THI", 16)                    # thi; later T1/TD/DT
            DXY = T("DXY", 4)                     # [dX | dY]
            DM = T("DM", 4)                       # [2dX | 2dY]
            ddS = T("ddS", 4)                     # [ddx | ddy]
            aP1, aP2 = T("aP1", 4), T("aP2", 4)
            EP = T("EP", 4)                       # [exP | eyP]
            u01 = sm.tile([P, SW], f16, tag="u01")
            union0 = sm.tile([P, FB], f16, tag="union0")
            UA = sm.tile([P, SW], f32, tag="UA")   # [union | area_c]
            area_c = UA[:, FB:SW]

            dXv = DXY[:, 0 * SW:1 * SW]
            dYv = DXY[:, 1 * SW:2 * SW]
            wcF = WC2[:, 0:SW]
            hcF = WC2[:, SW:2 * SW]
            wsF = WS2[:, 0:SW]
            hsF = WS2[:, SW:2 * SW]

            # ---- pre-pass, angle part (only needs angT) ----
            vtt(dlt[:], angV[:, 0], angV[:, 1], OP.subtract)     # a1-a2 (f32)
            S.activation(cS[:, 0:FB], angV[:, 1], AF.Sin, bias=pio2[:])   # c2
            S.activation(cS[:, FB:SW], angV[:, 0], AF.Sin, bias=pio2[:])  # c1
            S.activation(sS[:, 0:FB], angV[:, 1], AF.Sin)                 # s2
            S.activation(sS[:, FB:SW], angV[:, 0], AF.Sin)                # s1
            S.activation(SDS[:, 0:FB], dlt[:], AF.Sin)                    # sd
            S.activation(SDS[:, FB:SW], dlt[:], AF.Sin, scale=-1.0)      # -sd
            # cos(dlt) = sin(dlt + pi/2); wrap into [-pi, pi] first
            V.add_range_wrap(dltw[:], dlt[:], PI_2, 3.141592653589793,
                             6.283185307179586)
            S.activation(cdF[:], dltw[:], AF.Sin)                # cd (1 half)
            # scaled trig copies carry geometry scale + xy dequant into dX/dY
            S.activation(csS[:], cS[:], AF.Copy, scale=XSC)
            S.activation(ssS[:], sS[:], AF.Copy, scale=XSC)

            # ---- pre-pass, wh part ----
            vts(WH[:, 0:SW], whV[:, 0:2], 0.5 * SC, None, OP.mult)       # whS
            vts(WH[:, SW:2 * SW], whV[:, 2:4], 0.5 * SC, None, OP.mult)  # hhS
            WH3 = v2(WH[:])
            # [wc|hc] = [whS|hhS]*cd ; [ws|hs] = [whS|hhS]*sd
            cdb = bass.AP(cdF[:].tensor, cdF[:].offset,
                          [list(cdF[:].ap[0]), [0, 2], [0, 2], [1, FB]])
            WH4 = WH[:].rearrange("p (c h f) -> p c h f", c=2, h=2)
            vtt(WC2[:].rearrange("p (c h f) -> p c h f", c=2, h=2),
                WH4, cdb, OP.mult)
            vtt(v2(WS2[:]), WH3, bce(SDS[:]), OP.mult)
            # corner offsets: g0x = wc-hs, n1 = wc+hs, g0y = ws+hc, n2 = hc-ws
            vtt(GN[:, 0:SW], wcF, hsF, OP.subtract)
            vtt(GN[:, SW:2 * SW], wcF, hsF, OP.add)
            vtt(GN[:, 2 * SW:3 * SW], wsF, hcF, OP.add)
            vtt(GN[:, 3 * SW:4 * SW], hcF, wsF, OP.subtract)
            # clip half-extents [Wc|Hc] = half-swapped views of WH (no ops)
            whp = WH[:].ap[0]
            WHcF = bass.AP(WH[:].tensor, WH[:].offset + FB,
                           [list(whp), [SW, 2], [-FB, 2], [1, FB]])
            WcB = bass.AP(WH[:].tensor, WH[:].offset + FB,
                          [list(whp), [0, 2], [-FB, 2], [1, FB]])
            HcB = bass.AP(WH[:].tensor, WH[:].offset + SW + FB,
                          [list(whp), [0, 2], [-FB, 2], [1, FB]])
            # moving-box bbox half-extents: ex = |wc|+|hs|, ey = |ws|+|hc|
            S.activation(aP1[:], WC2[:], AF.Abs)   # [|wc| | |hc|]
            S.activation(aP2[:], WS2[:], AF.Abs)   # [|ws| | |hs|]
            vtt(EP[:, 0:SW], aP1[:, 0:SW], aP2[:, SW:2 * SW], OP.add)
            vtt(EP[:, SW:2 * SW], aP2[:, 0:SW], aP1[:, SW:2 * SW], OP.add)
            # negated-reciprocal planes rN = -1/d: rNX = [+1/(2wc) | -1/(2hs)],
            # rNY = [+1/(2ws) | +1/(2hc)]; staged f32 as [wc|hs|ws|hc], one
            # wide fast-reciprocal, clamped to +-CL in fp16.
            vts(RP[:, 0:SW], wcF, 2.0, 1e-20, OP.mult, OP.add)
            vts(RP[:, SW:2 * SW], hsF, -2.0, -1e-20, OP.mult, OP.add)
            vts(RP[:, 2 * SW:3 * SW], wsF, 2.0, 1e-20, OP.mult, OP.add)
            vts(RP[:, 3 * SW:4 * SW], hcF, 2.0, 1e-20, OP.mult, OP.add)
            V.reciprocal_approx_fast(out=RP[:], in_=RP[:])
            vts(RN[:], RP[:], CL, -CL, OP.min, OP.max)
            # half-window sizes |Wc*rN| per axis (abs on ACT)
            vtt(MW[:, 0:2 * SW].rearrange("p (e h f) -> p e h f", e=2, h=2),
                WcB, v2(RN[:, 0:2 * SW]).rearrange("p e (h f) -> p e h f",
                                                   h=2), OP.mult)
            vtt(MW[:, 2 * SW:4 * SW].rearrange("p (e h f) -> p e h f",
                                               e=2, h=2),
                HcB, v2(RN[:, 2 * SW:4 * SW]).rearrange("p e (h f) -> p e h f",
                                                        h=2), OP.mult)
            S.activation(AW[:], MW[:], AF.Abs)
            # union0 = (w1h1 + w2h2)/1024; *4 to /256 folded into union STT
            vtt(u01[:], WH[:, 0:SW], WH[:, SW:2 * SW], OP.mult)
            u013 = v2(u01[:])
            vtt(union0[:], u013[:, 0], u013[:, 1], OP.add)

            # ---- pre-pass, xy part (lands last) ----
            dd3 = ddS[:].rearrange("p (c h f) -> p c h f", c=2, h=2)
            ddc = ddS[:].rearrange("p (c f) -> p c f", c=2)
            vtt(dd3[:, 0, 0], xyV[:, 0], xyV[:, 1], OP.subtract)  # x1-x2
            S.activation(dd3[:, 0, 1], dd3[:, 0, 0], AF.Copy, scale=-1.0)
            vtt(dd3[:, 1, 0], xyV[:, 2], xyV[:, 3], OP.subtract)
            S.activation(dd3[:, 1, 1], dd3[:, 1, 0], AF.Copy, scale=-1.0)
            # delta = R^T * (center difference)/16:
            # aP1 = [csS*ddx | csS*ddy], aP2 = [ssS*ddx | ssS*ddy]
            vtt(v2(aP1[:]), bce(csS[:]), ddc, OP.mult)
            vtt(v2(aP2[:]), bce(ssS[:]), ddc, OP.mult)
            vtt(dXv, aP1[:, 0:SW], aP2[:, SW:2 * SW], OP.add)
            vtt(dYv, aP1[:, SW:2 * SW], aP2[:, 0:SW], OP.subtract)
            S.activation(DM[:, 0:SW], dXv, AF.Copy, scale=2.0)
            S.activation(DM[:, SW:2 * SW], dYv, AF.Copy, scale=2.0)

            # corners, edges 0,1 only: ax0 = dX+g0x, ax1 = dX-n1,
            # ay0 = dY+g0y, ay1 = dY+n2
            vtt(AXY[:, 0:SW], dXv, GN[:, 0:SW], OP.add)
            vtt(AXY[:, SW:2 * SW], dXv, GN[:, SW:2 * SW], OP.subtract)
            vtt(AXY[:, 2 * SW:3 * SW], dYv, GN[:, 2 * SW:3 * SW], OP.add)
            vtt(AXY[:, 3 * SW:4 * SW], dYv, GN[:, 3 * SW:4 * SW], OP.add)

            # input tiles no longer needed: free the io pool
            stack.close()

            # ---- slab roots, center form: c01 = ax*rN; the e23 interval
            # bounds derive from the e01 bounds by the point-symmetry shift:
            # tlo23 = tlo01 - mm, thi23 = thi01 - mm (mm = 2dX*rN), so c23
            # is never materialized and aw is applied once.
            vtt(CXY[:, 0:4 * SW], AXY[:], RN[:], OP.mult)
            dmb = bass.AP(DM[:].tensor, DM[:].offset,
                          [list(DM[:].ap[0]), [SW, 2], [0, 2], [1, SW]])
            MM = CXY[:, 4 * SW:8 * SW]
            vtt(MM.rearrange("p (a e f) -> p a e f", a=2, e=2), dmb,
                RN[:].rearrange("p (a e f) -> p a e f", a=2, e=2), OP.mult)
            vtt(TLO[:, 0:4 * SW], CXY[:, 0:4 * SW], AW[:], OP.subtract)
            vtt(THI[:, 0:4 * SW], CXY[:, 0:4 * SW], AW[:], OP.add)
            vtt(TLO[:, 4 * SW:8 * SW], TLO[:, 0:4 * SW], MM, OP.subtract)
            vtt(THI[:, 4 * SW:8 * SW], THI[:, 0:4 * SW], MM, OP.subtract)
            # interval intersect across axes, clamp to [0,1], dt = relu(t1-t0)
            # T0 lives in TLO[0:4SW], T1/TD in THI[0:4SW], DT in THI[4SW:8SW]
            tlo4 = TLO[:].rearrange("p (g a f) -> p g a f", g=2, a=2)
            thi4 = THI[:].rearrange("p (g a f) -> p g a f", g=2, a=2)
            T0 = TLO[:, 0:4 * SW]
            T0v = tlo4[:, :, 0]
            vtt(T0v, tlo4[:, :, 0], tlo4[:, :, 1], OP.max)
            vts(T0v, T0v, 0.0, None, OP.max)
            T1v = thi4[:, :, 0]
            vtt(T1v, thi4[:, :, 0], thi4[:, :, 1], OP.min)
            vts(T1v, T1v, 1.0, None, OP.min)
            vtt(T1v, T1v, T0v, OP.subtract)                   # td in place
            # dt = relu(td) lands contiguous in MW (free after the AW abs)
            DT = MW[:]
            S.activation(v2(DT), T1v, AF.Relu)
            dtg = DT.rearrange("p (g e h f) -> p g e h f", g=2, e=2, h=2)

            # ---- inter via rebased origins: the h0 half's per-edge cad is
            # the constant u01 (origin at the moving box's own center), so
            # inter = u01*sum(dt) per half + the h1 cross terms
            # sum dt over pair and edge dims -> S_dt per (h, box)
            vtt(AXY[:, 0:2 * SW], DT[:, 0:2 * SW], DT[:, 2 * SW:4 * SW],
                OP.add)
            sdt = sm.tile([P, SW], f16, tag="sdt")
            vtt(sdt[:], AXY[:, 0:SW], AXY[:, SW:2 * SW], OP.add)
            bse = sm.tile([P, SW], f16, tag="bse")
            vtt(bse[:], u01[:], sdt[:], OP.mult)
            bse3 = v2(bse[:])
            # h1 cross terms: crA = dY*wc - dX*ws, crBn = dX*hc + dY*hs
            dX_h1 = DXY[:, FB:SW]
            dY_h1 = DXY[:, SW + FB:2 * SW]
            wc_h1 = WC2[:, FB:SW]
            hc_h1 = WC2[:, SW + FB:2 * SW]
            ws_h1 = WS2[:, FB:SW]
            hs_h1 = WS2[:, SW + FB:2 * SW]
            crA = sm.tile([P, FB], f16, tag="crA")
            crBn = sm.tile([P, FB], f16, tag="crBn")
            st1 = sm.tile([P, FB], f16, tag="st1")
            st2 = sm.tile([P, FB], f16, tag="st2")
            vtt(crA[:], dY_h1, wc_h1, OP.mult)
            vtt(st1[:], dX_h1, ws_h1, OP.mult)
            vtt(crA[:], crA[:], st1[:], OP.subtract)
            vtt(crBn[:], dX_h1, hc_h1, OP.mult)
            vtt(st1[:], dY_h1, hs_h1, OP.mult)
            vtt(crBn[:], crBn[:], st1[:], OP.add)
            # du = dt_e0 - dt_e2, dv = dt_e1 - dt_e3 (h1 planes)
            vtt(st1[:], dtg[:, 0, 0, 1], dtg[:, 1, 0, 1], OP.subtract)
            vtt(st2[:], dtg[:, 0, 1, 1], dtg[:, 1, 1, 1], OP.subtract)
            vtt(crA[:], crA[:], st1[:], OP.mult)
            vtt(crBn[:], crBn[:], st2[:], OP.mult)
            inter16 = sm.tile([P, FB], f16, tag="inter16")
            vtt(inter16[:], bse3[:, 0], bse3[:, 1], OP.add)
            vtt(inter16[:], inter16[:], crA[:], OP.add)
            vtt(inter16[:], inter16[:], crBn[:], OP.subtract)

            # ---- enclosing rect (bbox in each frame, min of the two) ----
            # scratch inside CXY (dead after the tlo/thi ops)
            ES1 = CXY[:, 0:2 * SW]
            ES2 = CXY[:, 2 * SW:4 * SW]
            EXT = CXY[:, 4 * SW:6 * SW]
            vtt(ES1, DXY[:, 0:2 * SW], EP[:], OP.add)
            vtt(ES2, EP[:], DXY[:, 0:2 * SW], OP.subtract)
            vtt(v2(ES1, h=2).rearrange("p a (h f) -> p a h f", h=2), 
                v2(ES1, h=2).rearrange("p a (h f) -> p a h f", h=2),
                WHcF, OP.max)
            vtt(v2(ES2, h=2).rearrange("p a (h f) -> p a h f", h=2),
                v2(ES2, h=2).rearrange("p a (h f) -> p a h f", h=2),
                WHcF, OP.max)
            vtt(EXT, ES1, ES2, OP.add)
            exs = sm.tile([P, SW], f16, tag="exs")
            vtt(exs[:], EXT[:, 0:SW], EXT[:, SW:2 * SW], OP.mult)
            es3 = v2(exs[:])
            vtt(area_c, es3[:, 0], es3[:, 1], OP.min)

            inter = sm.tile([P, FB], f32, tag="inter")
            vts(inter[:], inter16[:], 0.0, None, OP.max)  # inter area (/256)

            # ---- final loss (fp32), cubes via one packed ACT Square ----
            fr1 = sm.tile([P, SW], f32, tag="fr1")
            IR = sm.tile([P, SW], f16, tag="IR")       # [iou | rr]
            SQ = sm.tile([P, SW], f16, tag="SQ")
            GU = sm.tile([P, FB], f16, tag="GU")
            lsa = sm.tile([P, 1], f32, tag="lsa")
            union = UA[:, 0:FB]
            # union = 4*union0 - inter  (the *4 restores the /256 scale)
            V.scalar_tensor_tensor(union, union0[:], 4.0, inter[:],
                                   OP.mult, OP.subtract)
            V.reciprocal_approx_fast(out=fr1[:], in_=UA[:])
            vtt(IR[:, 0:FB], inter[:], fr1[:, 0:FB], OP.mult)
            vts(IR[:, 0:FB], IR[:, 0:FB], 1e-6, 1.0, OP.max, OP.min)
            vtt(fr1[:, FB:SW], union, fr1[:, FB:SW], OP.mult)
            vts(fr1[:, FB:SW], fr1[:, FB:SW], 0.0, 1.0, OP.max, OP.min)
            vts(IR[:, FB:SW], fr1[:, FB:SW], -1.0, 1.0, OP.mult, OP.add)
            vtt(SQ[:], IR[:], IR[:], OP.mult)
            vtt(SQ[:], SQ[:], IR[:], OP.mult)                      # cubes
            cb3 = v2(SQ[:])
            vtt(GU[:], cb3[:, 0], cb3[:, 1], OP.subtract)          # giou
            V.tensor_reduce(lsa[:], GU[:], AXL.X, OP.add)          # sum giou
            if debug:
                nc.sync.dma_start(out=dbg_d[0], in_=GU[:])
                nc.sync.dma_start(out=dbg_d[1], in_=inter[:])
                nc.sync.dma_start(out=dbg_d[2], in_=union)
                nc.sync.dma_start(out=dbg_d[3], in_=area_c)
            nc.sync.dma_start(out=out_d[:], in_=lsa[:])

    nc.finalize()
    return nc


def _get_nc():
    if "nc" not in _CACHE:
        _CACHE["nc"] = _build()
    return _CACHE["nc"]


def _repack(pred, target):
    """Per-core input repack: planar rows so every SBUF slice is packed.
    ang/wh in fp16; xy quantized to int16 units of 1/32 px (diffs <= ~1500
    units stay exact in fp16). Rows beyond N_CORE are padded with concentric
    axis-aligned boxes whose giou is exactly 1/64 (subtracted on the host)."""
    in_maps = []
    for i in range(N_CORES):
        sl = slice(i * N_CORE, (i + 1) * N_CORE)
        p, t = pred[sl], target[sl]
        ang = np.zeros((2, NPAD), np.float16)
        ang[0, :N_CORE] = p[:, 4]
        ang[1, :N_CORE] = t[:, 4]
        wh = np.empty((4, NPAD), np.float16)
        wh[0, N_CORE:] = 16.0
        wh[1, N_CORE:] = 8.0
        wh[2, N_CORE:] = 16.0
        wh[3, N_CORE:] = 8.0
        wh[0, :N_CORE] = p[:, 2]
        wh[1, :N_CORE] = t[:, 2]
        wh[2, :N_CORE] = p[:, 3]
        wh[3, :N_CORE] = t[:, 3]
        xy = np.full((4, NPAD), 16384, np.int16)
        for r, col in enumerate((p[:, 0], t[:, 0], p[:, 1], t[:, 1])):
            xy[r, :N_CORE] = np.clip(np.rint(col * XQ), 0, 32767).astype(np.int16)

        def lay(a):
            k = a.shape[0]
            return np.ascontiguousarray(
                a.reshape(k, P, FB).transpose(1, 0, 2).reshape(P, k * FB))
        in_maps.append({"ang": lay(ang), "wh": lay(wh), "xy": lay(xy)})
    return in_maps


def kernel(pred, target):
    from concourse.bass_utils import run_bass_kernel_spmd

    pred = np.ascontiguousarray(np.asarray(pred, dtype=np.float32))
    target = np.ascontiguousarray(np.asarray(target, dtype=np.float32))
    nc = _get_nc()
    in_maps = _repack(pred, target)
    res = run_bass_kernel_spmd(nc, in_maps, core_ids=list(range(N_CORES)))
    gsum = np.float64(0.0)
    for i in range(N_CORES):
        gsum += np.asarray(res.results[i]["out"], dtype=np.float64).sum()
    # subtract the exact giou (=1/64) of the concentric pad boxes
    gsum -= float((NPAD - N_CORE) * N_CORES) * 0.015625
    # loss = mean(1 - giou) = 1 - sum(giou)/N
    return np.float32(1.0 - gsum / N_TOTAL)


# revision 22
# speedup vs baseline: 1.0268x; 1.0023x over previous
"""AlphaRotatedGIoULoss on 8 TRN2 NeuronCores.

Data-parallel: 500000 box pairs sharded 62500/core, laid out as
(128 partitions x 489 boxes). Per-box rotated-GIoU via a branchless
line-integral intersection (slab clipping in each box's axis-aligned
frame + a frame-change correction term), so no sorting/gather is needed.

Restructured v2 (113us session baseline -> 87.8-89.4us, same-p-state;
device DVFS adds ~+20% run-to-run on throttled runs):
- slab roots in center +- half-window form: t = c -+ |Wc*r| with
  pre-negated reciprocal planes, killing the per-edge min/max and subs;
  the e23 interval bounds derive from the e01 bounds by the point-symmetry
  shift (tlo23 = tlo01 - mm), so c23 is never materialized and the
  half-window is applied once. x and y axes fused into 8u ops.
- cross(corner_e, dir_e)/2 = +-(cross(center,dir)/2) + wh/1024 (the wh
  term is the union's u01 tile), so the full corner planes e2/e3, the
  4SW direction planes, and the 8 ACT copies that built them are gone.
  Everything downstream runs at half-cad scale (final Relu scale 1.0).
- all four reciprocal planes merged into one wide RECIPROCAL_APPROX_FAST
  plus two clamp/cast passes; enclosing-rect x/y stacked into 4u ops.
- the h0 half's line integral is rebased to the moving box's center, so
  its per-edge cad is the constant u01: the whole SA-correction section
  and the per-edge cad/pieces multiplies collapse into
  inter = u01*sum(dt) per half + two h1 cross terms.
- tail packs [union|area_c] into one fast-reciprocal and [iou|rr] into
  one fp16 cube chain; iou/rr clamped to [0,1] so the few near-degenerate
  slab boxes (fp16-saturated reciprocal planes) stay bounded.
Heavy chain is fp16 (DVE 2x mode), geometry pre-scaled by 1/16; scratch
tiles are re-used across phases to stay inside SBUF.
"""
import sys
import numpy as np

for _p in ("/opt/trn_rl_repo", "/root/.axon_site/_ro/trn_rl_repo"):
    if _p not in sys.path:
        sys.path.insert(0, _p)

N_CORES = 8
N_TOTAL = 500000
N_CORE = N_TOTAL // N_CORES   # 62500
P = 128                       # all partitions
FB = 489                      # boxes per partition row (128*489 = 62592)
NPAD = P * FB                 # per-core padded count (92 identity pad boxes)
SW = 2 * FB                   # stacked width (both halves)
PI_2 = 1.5707963267948966
SC = 1.0 / 16.0               # global geometry scale (power of 2, exact)
XQ = 32.0                     # xy fixed-point scale (int16 units = px/32)
XSC = SC / XQ                 # folds the xy dequant into the trig scale
CL = 30000.0                  # fp16-safe clamp for reciprocal planes

_CACHE = {}


def _build():
    import concourse.bass as bass
    import concourse.bacc as bacc
    import concourse.tile as tile
    from concourse import mybir

    f32 = mybir.dt.float32
    f16 = mybir.dt.float16
    i16 = mybir.dt.int16
    AF = mybir.ActivationFunctionType
    OP = mybir.AluOpType
    AXL = mybir.AxisListType
    import os
    debug = bool(os.environ.get("K_DEBUG"))
    nc = bacc.Bacc(None, target_bir_lowering=False)
    ang_d = nc.declare_dram_parameter("ang", [P, 2 * FB], f16, isOutput=False)
    wh_d = nc.declare_dram_parameter("wh", [P, 4 * FB], f16, isOutput=False)
    xy_d = nc.declare_dram_parameter("xy", [P, 4 * FB], i16, isOutput=False)
    out_d = nc.declare_dram_parameter("out", [P, 1], f32, isOutput=True)
    dbg_d = None
    if debug:
        dbg_d = nc.declare_dram_parameter("dbg", [4, P, FB], f32, isOutput=True)

    V = nc.vector
    S = nc.scalar

    def vtt(out, a, b, op):
        V.tensor_tensor(out, a, b, op)

    def vts(out, in_, s1, s2, op0, op1=None):
        if op1 is None:
            V.tensor_scalar(out, in_, s1, None, op0)
        else:
            V.tensor_scalar(out, in_, s1, s2, op0, op1)

    def bce(apv, n=2, axis=1):
        # stride-0 broadcast: insert a [0, n] dim at `axis` (after partition)
        ap_l = [list(d) for d in apv.ap]
        ap_l.insert(axis, [0, n])
        return bass.AP(apv.tensor, apv.offset, ap_l)

    def v2(ap, h=2):
        return ap.rearrange("p (h f) -> p h f", h=h)

    from contextlib import ExitStack

    with tile.TileContext(nc) as tc:
        with (
            tc.tile_pool(name="pre", bufs=1) as pre,
            tc.tile_pool(name="small", bufs=1) as sm,
            ExitStack() as stack,
        ):
            io = stack.enter_context(tc.tile_pool(name="io", bufs=1))
            angT = io.tile([P, 2 * FB], f16, tag="angT")
            whT = io.tile([P, 4 * FB], f16, tag="whT")
            xyT = io.tile([P, 4 * FB], i16, tag="xyT")
            pio2 = sm.tile([P, 1], f32, tag="pio2")
            V.memset(pio2[:], PI_2)
            angV = angT[:].rearrange("p (h f) -> p h f", h=2)
            whV = whT[:].rearrange("p (c f) -> p c f", c=4)   # w1,w2,h1,h2
            xyV = xyT[:].rearrange("p (c f) -> p c f", c=4)   # x1,x2,y1,y2
            # host pre-shuffles inputs into these exact SBUF layouts, so each
            # partition line is one fully-contiguous DMA descriptor.
            # angles first (small, unblocks the Sin chain), then wh, then xy
            nc.sync.dma_start(out=angT[:], in_=ang_d[:])
            nc.sync.dma_start(out=whT[:], in_=wh_d[:])
            nc.sync.dma_start(out=xyT[:], in_=xy_d[:])
            # 1-elem warm-up: loads the Sin ACT table while the DMA runs
            warm = sm.tile([P, 1], f32, tag="warm")
            S.activation(warm[:], pio2[:], AF.Sin)

            def T(name, units, dt=f16):
                # `units` in FB-widths
                return pre.tile([P, units * FB], dt, name=name, tag=name)

            # --- tiles (persistent + phase-reused scratch) ---
            dlt, dltw = T("dlt", 1), T("dltw", 1)
            cdF = T("cdF", 1)                     # cos(dlt), one half
            SDS = T("SDS", 2)                     # [sd | -sd]
            cS, sS = T("cS", 2), T("sS", 2)       # [c2|c1], [s2|s1]
            csS, ssS = T("csS", 2), T("ssS", 2)
            WH = T("WH", 4)                       # [whS | hhS]
            WC2 = T("WC2", 4)                     # [wc | hc]
            WS2 = T("WS2", 4)                     # [ws | hs]
            GN = T("GN", 8)                       # [g0x | n1 | g0y | n2]
            AXY = T("AXY", 8)                     # corners [ax0|ax1|ay0|ay1]
            RP = T("RP", 8, f32)                  # recip staging [wc|hs|ws|hc]
            RN = T("RN", 8)                       # [rNX(e0,e1) | rNY(e0,e1)]
            MW = T("MW", 8)                       # [Wc*rNX | Hc*rNY] -> scratch
            AW = T("AW", 8)                       # |MW|
            CXY = T("CXY", 16)                    # [c01xy | c23xy]
            TLO = T("TLO", 16)                    # tlo; later T0/CAD/PC
            THI = T("THI", 16)                    # thi; later T1/TD/DT
            DXY = T("DXY", 4)                     # [dX | dY]
            DM = T("DM", 4)                       # [2dX | 2dY]
            ddS = T("ddS", 4)                     # [ddx | ddy]
            aP1, aP2 = T("aP1", 4), T("aP2", 4)
            EP = T("EP", 4)                       # [exP | eyP]
            u01 = sm.tile([P, SW], f16, tag="u01")
            union0 = sm.tile([P, FB], f16, tag="union0")
            UA = sm.tile([P, SW], f32, tag="UA")   # [union | area_c]
            area_c = UA[:, FB:SW]

            dXv = DXY[:, 0 * SW:1 * SW]
            dYv = DXY[:, 1 * SW:2 * SW]
            wcF = WC2[:, 0:SW]
            hcF = WC2[:, SW:2 * SW]
            wsF = WS2[:, 0:SW]
            hsF = WS2[:, SW:2 * SW]

            # ---- pre-pass, angle part (only needs angT) ----
            vtt(dlt[:], angV[:, 0], angV[:, 1], OP.subtract)     # a1-a2 (f32)
            S.activation(cS[:, 0:FB], angV[:, 1], AF.Sin, bias=pio2[:])   # c2
            S.activation(cS[:, FB:SW], angV[:, 0], AF.Sin, bias=pio2[:])  # c1
            S.activation(sS[:, 0:FB], angV[:, 1], AF.Sin)                 # s2
            S.activation(sS[:, FB:SW], angV[:, 0], AF.Sin)                # s1
            S.activation(SDS[:, 0:FB], dlt[:], AF.Sin)                    # sd
            S.activation(SDS[:, FB:SW], dlt[:], AF.Sin, scale=-1.0)      # -sd
            # cos(dlt) = sin(dlt + pi/2); wrap into [-pi, pi] first
            V.add_range_wrap(dltw[:], dlt[:], PI_2, 3.141592653589793,
                             6.283185307179586)
            S.activation(cdF[:], dltw[:], AF.Sin)                # cd (1 half)
            # scaled trig copies carry geometry scale + xy dequant into dX/dY
            S.activation(csS[:], cS[:], AF.Copy, scale=XSC)
            S.activation(ssS[:], sS[:], AF.Copy, scale=XSC)

            # ---- pre-pass, wh part ----
            vts(WH[:, 0:SW], whV[:, 0:2], 0.5 * SC, None, OP.mult)       # whS
            vts(WH[:, SW:2 * SW], whV[:, 2:4], 0.5 * SC, None, OP.mult)  # hhS
            WH3 = v2(WH[:])
            # [wc|hc] = [whS|hhS]*cd ; [ws|hs] = [whS|hhS]*sd
            cdb = bass.AP(cdF[:].tensor, cdF[:].offset,
                          [list(cdF[:].ap[0]), [0, 2], [0, 2], [1, FB]])
            WH4 = WH[:].rearrange("p (c h f) -> p c h f", c=2, h=2)
            vtt(WC2[:].rearrange("p (c h f) -> p c h f", c=2, h=2),
                WH4, cdb, OP.mult)
            vtt(v2(WS2[:]), WH3, bce(SDS[:]), OP.mult)
            # corner offsets: g0x = wc-hs, n1 = wc+hs, g0y = ws+hc, n2 = hc-ws
            vtt(GN[:, 0:SW], wcF, hsF, OP.subtract)
            vtt(GN[:, SW:2 * SW], wcF, hsF, OP.add)
            vtt(GN[:, 2 * SW:3 * SW], wsF, hcF, OP.add)
            vtt(GN[:, 3 * SW:4 * SW], hcF, wsF, OP.subtract)
            # clip half-extents [Wc|Hc] = half-swapped views of WH (no ops)
            whp = WH[:].ap[0]
            WHcF = bass.AP(WH[:].tensor, WH[:].offset + FB,
                           [list(whp), [SW, 2], [-FB, 2], [1, FB]])
            WcB = bass.AP(WH[:].tensor, WH[:].offset + FB,
                          [list(whp), [0, 2], [-FB, 2], [1, FB]])
            HcB = bass.AP(WH[:].tensor, WH[:].offset + SW + FB,
                          [list(whp), [0, 2], [-FB, 2], [1, FB]])
            # moving-box bbox half-extents: ex = |wc|+|hs|, ey = |ws|+|hc|
            S.activation(aP1[:], WC2[:], AF.Abs)   # [|wc| | |hc|]
            S.activation(aP2[:], WS2[:], AF.Abs)   # [|ws| | |hs|]
            vtt(EP[:, 0:SW], aP1[:, 0:SW], aP2[:, SW:2 * SW], OP.add)
            vtt(EP[:, SW:2 * SW], aP2[:, 0:SW], aP1[:, SW:2 * SW], OP.add)
            # negated-reciprocal planes rN = -1/d: rNX = [+1/(2wc) | -1/(2hs)],
            # rNY = [+1/(2ws) | +1/(2hc)]; staged f32 as [wc|hs|ws|hc], one
            # wide fast-reciprocal, clamped to +-CL in fp16.
            vts(RP[:, 0:SW], wcF, 2.0, 1e-20, OP.mult, OP.add)
            vts(RP[:, SW:2 * SW], hsF, -2.0, -1e-20, OP.mult, OP.add)
            vts(RP[:, 2 * SW:3 * SW], wsF, 2.0, 1e-20, OP.mult, OP.add)
            vts(RP[:, 3 * SW:4 * SW], hcF, 2.0, 1e-20, OP.mult, OP.add)
            V.reciprocal_approx_fast(out=RP[:], in_=RP[:])
            vts(RN[:], RP[:], CL, -CL, OP.min, OP.max)
            # half-window sizes |Wc*rN| per axis (abs on ACT)
            vtt(MW[:, 0:2 * SW].rearrange("p (e h f) -> p e h f", e=2, h=2),
                WcB, v2(RN[:, 0:2 * SW]).rearrange("p e (h f) -> p e h f",
                                                   h=2), OP.mult)
            vtt(MW[:, 2 * SW:4 * SW].rearrange("p (e h f) -> p e h f",
                                               e=2, h=2),
                HcB, v2(RN[:, 2 * SW:4 * SW]).rearrange("p e (h f) -> p e h f",
                                                        h=2), OP.mult)
            S.activation(AW[:], MW[:], AF.Abs)
            # union0 = (w1h1 + w2h2)/1024; *4 to /256 folded into union STT
            vtt(u01[:], WH[:, 0:SW], WH[:, SW:2 * SW], OP.mult)
            u013 = v2(u01[:])
            vtt(union0[:], u013[:, 0], u013[:, 1], OP.add)

            # ---- pre-pass, xy part (lands last) ----
            dd3 = ddS[:].rearrange("p (c h f) -> p c h f", c=2, h=2)
            ddc = ddS[:].rearrange("p (c f) -> p c f", c=2)
            vtt(dd3[:, 0, 0], xyV[:, 0], xyV[:, 1], OP.subtract)  # x1-x2
            S.activation(dd3[:, 0, 1], dd3[:, 0, 0], AF.Copy, scale=-1.0)
            vtt(dd3[:, 1, 0], xyV[:, 2], xyV[:, 3], OP.subtract)
            S.activation(dd3[:, 1, 1], dd3[:, 1, 0], AF.Copy, scale=-1.0)
            # delta = R^T * (center difference)/16:
            # aP1 = [csS*ddx | csS*ddy], aP2 = [ssS*ddx | ssS*ddy]
            vtt(v2(aP1[:]), bce(csS[:]), ddc, OP.mult)
            vtt(v2(aP2[:]), bce(ssS[:]), ddc, OP.mult)
            vtt(dXv, aP1[:, 0:SW], aP2[:, SW:2 * SW], OP.add)
            vtt(dYv, aP1[:, SW:2 * SW], aP2[:, 0:SW], OP.subtract)
            S.activation(DM[:, 0:SW], dXv, AF.Copy, scale=2.0)
            S.activation(DM[:, SW:2 * SW], dYv, AF.Copy, scale=2.0)

            # corners, edges 0,1 only: ax0 = dX+g0x, ax1 = dX-n1,
            # ay0 = dY+g0y, ay1 = dY+n2
            vtt(AXY[:, 0:SW], dXv, GN[:, 0:SW], OP.add)
            vtt(AXY[:, SW:2 * SW], dXv, GN[:, SW:2 * SW], OP.subtract)
            vtt(AXY[:, 2 * SW:3 * SW], dYv, GN[:, 2 * SW:3 * SW], OP.add)
            vtt(AXY[:, 3 * SW:4 * SW], dYv, GN[:, 3 * SW:4 * SW], OP.add)

            # input tiles no longer needed: free the io pool
            stack.close()

            # ---- slab roots, center form: c01 = ax*rN; the e23 interval
            # bounds derive from the e01 bounds by the point-symmetry shift:
            # tlo23 = tlo01 - mm, thi23 = thi01 - mm (mm = 2dX*rN), so c23
            # is never materialized and aw is applied once.
            vtt(CXY[:, 0:4 * SW], AXY[:], RN[:], OP.mult)
            dmb = bass.AP(DM[:].tensor, DM[:].offset,
                          [list(DM[:].ap[0]), [SW, 2], [0, 2], [1, SW]])
            MM = CXY[:, 4 * SW:8 * SW]
            vtt(MM.rearrange("p (a e f) -> p a e f", a=2, e=2), dmb,
                RN[:].rearrange("p (a e f) -> p a e f", a=2, e=2), OP.mult)
            vtt(TLO[:, 0:4 * SW], CXY[:, 0:4 * SW], AW[:], OP.subtract)
            vtt(THI[:, 0:4 * SW], CXY[:, 0:4 * SW], AW[:], OP.add)
            vtt(TLO[:, 4 * SW:8 * SW], TLO[:, 0:4 * SW], MM, OP.subtract)
            vtt(THI[:, 4 * SW:8 * SW], THI[:, 0:4 * SW], MM, OP.subtract)
            # interval intersect across axes, clamp to [0,1], dt = relu(t1-t0)
            # T0 lives in TLO[0:4SW], T1/TD in THI[0:4SW], DT in THI[4SW:8SW]
            tlo4 = TLO[:].rearrange("p (g a f) -> p g a f", g=2, a=2)
            thi4 = THI[:].rearrange("p (g a f) -> p g a f", g=2, a=2)
            T0 = TLO[:, 0:4 * SW]
            T0v = tlo4[:, :, 0]
            vtt(T0v, tlo4[:, :, 0], tlo4[:, :, 1], OP.max)
            vts(T0v, T0v, 0.0, None, OP.max)
            T1v = thi4[:, :, 0]
            vtt(T1v, thi4[:, :, 0], thi4[:, :, 1], OP.min)
            vts(T1v, T1v, 1.0, None, OP.min)
            vtt(T1v, T1v, T0v, OP.subtract)                   # td in place
            # dt = relu(td) lands contiguous in MW (free after the AW abs)
            DT = MW[:]
            S.activation(v2(DT), T1v, AF.Relu)
            dtg = DT.rearrange("p (g e h f) -> p g e h f", g=2, e=2, h=2)

            # ---- inter via rebased origins: the h0 half's per-edge cad is
            # the constant u01 (origin at the moving box's own center), so
            # inter = u01*sum(dt) per half + the h1 cross terms
            # sum dt over pair and edge dims -> S_dt per (h, box)
            vtt(AXY[:, 0:2 * SW], DT[:, 0:2 * SW], DT[:, 2 * SW:4 * SW],
                OP.add)
            sdt = sm.tile([P, SW], f16, tag="sdt")
            vtt(sdt[:], AXY[:, 0:SW], AXY[:, SW:2 * SW], OP.add)
            bse = sm.tile([P, SW], f16, tag="bse")
            vtt(bse[:], u01[:], sdt[:], OP.mult)
            bse3 = v2(bse[:])
            # h1 cross terms: crA = dY*wc - dX*ws, crBn = dX*hc + dY*hs
            dX_h1 = DXY[:, FB:SW]
            dY_h1 = DXY[:, SW + FB:2 * SW]
            wc_h1 = WC2[:, FB:SW]
            hc_h1 = WC2[:, SW + FB:2 * SW]
            ws_h1 = WS2[:, FB:SW]
            hs_h1 = WS2[:, SW + FB:2 * SW]
            crA = sm.tile([P, FB], f16, tag="crA")
            crBn = sm.tile([P, FB], f16, tag="crBn")
            st1 = sm.tile([P, FB], f16, tag="st1")
            st2 = sm.tile([P, FB], f16, tag="st2")
            vtt(crA[:], dY_h1, wc_h1, OP.mult)
            vtt(st1[:], dX_h1, ws_h1, OP.mult)
            vtt(crA[:], crA[:], st1[:], OP.subtract)
            vtt(crBn[:], dX_h1, hc_h1, OP.mult)
            vtt(st1[:], dY_h1, hs_h1, OP.mult)
            vtt(crBn[:], crBn[:], st1[:], OP.add)
            # du = dt_e0 - dt_e2, dv = dt_e1 - dt_e3 (h1 planes)
            vtt(st1[:], dtg[:, 0, 0, 1], dtg[:, 1, 0, 1], OP.subtract)
            vtt(st2[:], dtg[:, 0, 1, 1], dtg[:, 1, 1, 1], OP.subtract)
            vtt(crA[:], crA[:], st1[:], OP.mult)
            vtt(crBn[:], crBn[:], st2[:], OP.mult)
            inter16 = sm.tile([P, FB], f16, tag="inter16")
            vtt(inter16[:], bse3[:, 0], bse3[:, 1], OP.add)
            vtt(inter16[:], inter16[:], crA[:], OP.add)
            vtt(inter16[:], inter16[:], crBn[:], OP.subtract)

            # ---- enclosing rect (bbox in each frame, min of the two) ----
            # scratch inside CXY (dead after the tlo/thi ops)
            ES1 = CXY[:, 0:2 * SW]
            ES2 = CXY[:, 2 * SW:4 * SW]
            EXT = CXY[:, 4 * SW:6 * SW]
            vtt(ES1, DXY[:, 0:2 * SW], EP[:], OP.add)
            vtt(ES2, EP[:], DXY[:, 0:2 * SW], OP.subtract)
            vtt(v2(ES1, h=2).rearrange("p a (h f) -> p a h f", h=2), 
                v2(ES1, h=2).rearrange("p a (h f) -> p a h f", h=2),
                WHcF, OP.max)
            vtt(v2(ES2, h=2).rearrange("p a (h f) -> p a h f", h=2),
                v2(ES2, h=2).rearrange("p a (h f) -> p a h f", h=2),
                WHcF, OP.max)
            vtt(EXT, ES1, ES2, OP.add)
            exs = sm.tile([P, SW], f16, tag="exs")
            vtt(exs[:], EXT[:, 0:SW], EXT[:, SW:2 * SW], OP.mult)
            es3 = v2(exs[:])
            vtt(area_c, es3[:, 0], es3[:, 1], OP.min)

            # ---- final loss, cubes via one fp16 chain. The area_c
            # reciprocal runs early (hidden under the heavy section); the
            # iou clamp already bounds negative-inter degenerates so the
            # raw fp16 inter16 feeds union/iou directly.
            fr1 = sm.tile([P, SW], f32, tag="fr1")
            IR = sm.tile([P, SW], f16, tag="IR")       # [iou | rr]
            SQ = sm.tile([P, SW], f16, tag="SQ")
            GU = sm.tile([P, FB], f16, tag="GU")
            lsa = sm.tile([P, 1], f32, tag="lsa")
            union = UA[:, 0:FB]
            V.reciprocal_approx_fast(out=fr1[:, FB:SW], in_=area_c)
            # union = 4*union0 - inter  (the *4 restores the /256 scale)
            V.scalar_tensor_tensor(union, union0[:], 4.0, inter16[:],
                                   OP.mult, OP.subtract)
            V.reciprocal_approx_fast(out=fr1[:, 0:FB], in_=union)
            vtt(IR[:, 0:FB], inter16[:], fr1[:, 0:FB], OP.mult)
            vts(IR[:, 0:FB], IR[:, 0:FB], 1e-6, 1.0, OP.max, OP.min)
            vtt(fr1[:, FB:SW], union, fr1[:, FB:SW], OP.mult)
            vts(fr1[:, FB:SW], fr1[:, FB:SW], 0.0, 1.0, OP.max, OP.min)
            vts(IR[:, FB:SW], fr1[:, FB:SW], -1.0, 1.0, OP.mult, OP.add)
            vtt(SQ[:], IR[:], IR[:], OP.mult)
            vtt(SQ[:], SQ[:], IR[:], OP.mult)                      # cubes
            cb3 = v2(SQ[:])
            vtt(GU[:], cb3[:, 0], cb3[:, 1], OP.subtract)          # giou
            V.tensor_reduce(lsa[:], GU[:], AXL.X, OP.add)          # sum giou
            if debug:
                nc.sync.dma_start(out=dbg_d[0], in_=GU[:])
                nc.sync.dma_start(out=dbg_d[1], in_=inter16[:])
                nc.sync.dma_start(out=dbg_d[2], in_=union)
                nc.sync.dma_start(out=dbg_d[3], in_=area_c)
            nc.sync.dma_start(out=out_d[:], in_=lsa[:])

    nc.finalize()
    return nc


def _get_nc():
    if "nc" not in _CACHE:
        _CACHE["nc"] = _build()
    return _CACHE["nc"]


def _repack(pred, target):
    """Per-core input repack: planar rows so every SBUF slice is packed.
    ang/wh in fp16; xy quantized to int16 units of 1/32 px (diffs <= ~1500
    units stay exact in fp16). Rows beyond N_CORE are padded with concentric
    axis-aligned boxes whose giou is exactly 1/64 (subtracted on the host)."""
    in_maps = []
    for i in range(N_CORES):
        sl = slice(i * N_CORE, (i + 1) * N_CORE)
        p, t = pred[sl], target[sl]
        ang = np.zeros((2, NPAD), np.float16)
        ang[0, :N_CORE] = p[:, 4]
        ang[1, :N_CORE] = t[:, 4]
        wh = np.empty((4, NPAD), np.float16)
        wh[0, N_CORE:] = 16.0
        wh[1, N_CORE:] = 8.0
        wh[2, N_CORE:] = 16.0
        wh[3, N_CORE:] = 8.0
        wh[0, :N_CORE] = p[:, 2]
        wh[1, :N_CORE] = t[:, 2]
        wh[2, :N_CORE] = p[:, 3]
        wh[3, :N_CORE] = t[:, 3]
        xy = np.full((4, NPAD), 16384, np.int16)
        for r, col in enumerate((p[:, 0], t[:, 0], p[:, 1], t[:, 1])):
            xy[r, :N_CORE] = np.clip(np.rint(col * XQ), 0, 32767).astype(np.int16)

        def lay(a):
            k = a.shape[0]
            return np.ascontiguousarray(
                a.reshape(k, P, FB).transpose(1, 0, 2).reshape(P, k * FB))
        in_maps.append({"ang": lay(ang), "wh": lay(wh), "xy": lay(xy)})
    return in_maps


def kernel(pred, target):
    from concourse.bass_utils import run_bass_kernel_spmd

    pred = np.ascontiguousarray(np.asarray(pred, dtype=np.float32))
    target = np.ascontiguousarray(np.asarray(target, dtype=np.float32))
    nc = _get_nc()
    in_maps = _repack(pred, target)
    res = run_bass_kernel_spmd(nc, in_maps, core_ids=list(range(N_CORES)))
    gsum = np.float64(0.0)
    for i in range(N_CORES):
        gsum += np.asarray(res.results[i]["out"], dtype=np.float64).sum()
    # subtract the exact giou (=1/64) of the concentric pad boxes
    gsum -= float((NPAD - N_CORE) * N_CORES) * 0.015625
    # loss = mean(1 - giou) = 1 - sum(giou)/N
    return np.float32(1.0 - gsum / N_TOTAL)
